# revision 131
# baseline (speedup 1.0000x reference)
"""Transformer-XL multi-head self-attention on 8 Trainium2 NeuronCores.

Sharding: core c handles batch b = c//4 and heads {2*(c%4), 2*(c%4)+1}
(data-parallel over B x tensor-parallel over heads). Each core produces a
partial [N, E] output (its heads' w_o contributions); the host sums the 4
partials per batch element.

The XL relative-position term BD[i,j] = (q_i+v)·BDk[j-i+N-1] is computed
without the rel_shift gather via per-query rotation (angle-difference
identities): BD^T = Psi @ UW with Psi a shape-derived constant basis
(128 exact sin rows + 128 exact cos rows + 64 Chebyshev rows for the slow
frequencies) and UW per-query rotated coefficients.

Scores run on the PE in fp8e4 DoubleRow mode (0.5 cycles/row in the cost
model) with hi/lo error compensation: a bf16-accurate operand x is split
as x = hi + lo with hi = fp8(x), lo = fp8(x - hi), keeping selected cross
terms. Per 128-key tile the contraction is 6 chunks of 128 rows consumed
by 3 DoubleRow calls:
  [sin|cos]x[Uhi|Whi],
  [khi|Thi]x[qhi|chi], [klo|Thi]x[qhi|clo], [khi|Tlo]x[qlo|chi], pad
where T/c are the Chebyshev basis/coefficients and k/q carry the content
term (q+u)·k. The U/W (fast psi coefficient) lo-compensation is dropped
(one-sided both psi and U/W): host-side simulation puts the end-to-end
max-rel error at ~1.4% vs the 2% gate (vs ~1.2% with the compensation).
The value path (exp, V, attn@V, output projection) stays in bf16: fp8
noise there does not average out. exp is spread over Act/DVE/Pool
(Schraudolph on DVE/Pool; the extra Schraudolph noise is ~free: ~1.47%
even if every tile uses it).
"""

import sys

sys.path.insert(0, "/opt/trn_rl_repo")

import ml_dtypes
import numpy as np

import concourse.bass as bass
import concourse.mybir as mybir
from concourse import bacc
from concourse.masks import make_identity
from concourse.tile import TileContext

F32 = mybir.dt.float32
BF16 = mybir.dt.bfloat16
FP8 = mybir.dt.float8e4
I16 = mybir.dt.int16
AF = mybir.ActivationFunctionType
ALU = mybir.AluOpType
DR = mybir.MatmulPerfMode.DoubleRow

B, N, H, E, NH, D = 2, 2048, 2048, 512, 8, 64
HpN = H + N  # 4096
P = 128
NKT = HpN // P  # 32 key tiles
NPAIR = NKT // 2  # 16 key-tile pairs
NQC = N // 512  # 4 query chunks of 512
NEC = E // P  # 4 contraction chunks over E
NS = N // P  # 16 output row tiles
NT = 64  # chebyshev terms
HEADS_PER_CORE = 2
N_CORES = 8

LOG2E = 1.4426950408889634
SCORE_SHIFT = 1.5  # exp(s - c): cancels in softmax, bounds exp values
# exp tile engine rotation: (ctr % MOD) -> r < EXP_ACT on Act (exact),
# rest on DVE (Schraudolph). GPSIMD cannot read PSUM so Pool is out.
# Strict alternation: consecutive units' exps overlap across the two
# engines (each engine sees one ~1.1us exp per two 858ns PE units).
EXP_MOD, EXP_ACT = 2, 1


def build_program():
    nc = bacc.Bacc("TRN2", target_bir_lowering=False, debug=False)

    axT_d = nc.declare_dram_parameter("axT", [E, HpN], BF16, isOutput=False)
    rot_d = nc.declare_dram_parameter("rot", [E, N], BF16, isOutput=False)
    # SgF: shared fast-psi chunks, partition-major [p][t][c][j] so the DMA is
    # an identity layout with 8KB per-partition runs
    psiF_d = nc.declare_dram_parameter("psiF", [P, NKT * 2 * P], FP8, isOutput=False)
    # shared cheb T basis rows [p(64)][hi/lo][t][j]; identical for both heads
    # (placed at opposite partition halves on device)
    psiT_d = nc.declare_dram_parameter("psiT", [NT, 2 * NKT * P], FP8, isOutput=False)
    # fast-psi half-compensation stationary [cos_hi(f0:64)|sin_hi(f64:128)],
    # shared by both heads: [p][t][j]
    psiC_d = nc.declare_dram_parameter("psiC", [P, NKT * P], FP8, isOutput=False)
    sc_d = nc.declare_dram_parameter("sc", [2 * P, NT], BF16, isOutput=False)
    wq2_d = nc.declare_dram_parameter("wq2", [E, P], BF16, isOutput=False)
    wk2_d = nc.declare_dram_parameter("wk2", [E, P], BF16, isOutput=False)
    wv2_d = nc.declare_dram_parameter("wv2", [E, P], BF16, isOutput=False)
    wkrT_d = nc.declare_dram_parameter("wkrT", [P, E], BF16, isOutput=False)
    # wo duplicated on both partition halves (odd numT s-tiles live at 64:128)
    wo2_d = nc.declare_dram_parameter("wo2", [P, 2 * E], BF16, isOutput=False)
    ub2_d = nc.declare_dram_parameter("ub2", [P, 1], F32, isOutput=False)
    vb2_d = nc.declare_dram_parameter("vb2", [P, 1], F32, isOutput=False)
    # two per-head partial outputs (host sums): h0 streams during h1's
    # attention; h1 drains at the tail
    oA_d = nc.declare_dram_parameter("oA", [N, E], BF16, isOutput=True)
    oB_d = nc.declare_dram_parameter("oB", [N, E], BF16, isOutput=True)

    with TileContext(nc) as tc:
        with (
            tc.tile_pool(name="persist", bufs=1) as persist,
            tc.tile_pool(name="gst", bufs=4) as gst,       # G copies stream
            tc.tile_pool(name="mst", bufs=2) as mst,       # rotation temps
            tc.tile_pool(name="est", bufs=8) as est,       # exp tiles
            tc.tile_pool(name="dram", bufs=1, space="DRAM") as dram_pool,
            tc.tile_pool(name="pr", bufs=5, space="PSUM") as pr,   # 5x [P,512]
            tc.tile_pool(name="ph", bufs=1, space="PSUM") as ph,   # 3x [P,512]
        ):
            _sm = [0]

            def small_psum(shape, name, dtype=F32, tag=None):
                if tag is None:
                    i = _sm[0] % 3
                    _sm[0] += 1
                    tag = f"bank{i}"
                return ph.tile(shape, dtype, tag=tag, name=name)

            # ---------------- DMAs ----------------
            # One prioritized stream on the sync queue: the DMA engines are a
            # serialized resource, so emission order here IS the priority.
            # q proj needs {wq2, x-half}; the uw chain adds {wkr, rot, sc};
            # emit_k(4..7)/emit_v(x) add {wk2, wv2}; history keys come next,
            # then the attention-only psi tables and wo.
            wq2_s = persist.tile([P, NEC, P], BF16, tag="wq2")
            nc.sync.dma_start(wq2_s[:], wq2_d[:].rearrange("(c p) d -> p c d", p=P))
            # first x piece immediately after wq2 -- the small-weight DMAs'
            # per-transfer HWDGE overheads would delay the very first matmul
            axT_s = persist.tile([P, NEC, HpN], BF16, tag="axT", name="axT")
            axT = [axT_s[:, c, :] for c in range(NEC)]

            def x_piece(r):
                ks = slice(H + r * 512, H + (r + 1) * 512)
                nc.sync.dma_start(
                    axT_s[:, :, ks],
                    axT_d[:, ks].rearrange("(c p) k -> p c k", p=P),
                )

            x_piece(0)
            ub_s = persist.tile([P, 1], F32, tag="ub")
            nc.sync.dma_start(ub_s[:], ub2_d[:])
            vb_s = persist.tile([P, 1], F32, tag="vb")
            nc.sync.dma_start(vb_s[:], vb2_d[:])
            # wkr stacked on partitions: rows 0:64 = head0 d, 64:128 = head1 d
            wkr_s = persist.tile([P, NEC, P], BF16, tag="wkr")
            nc.sync.dma_start(
                wkr_s[:], wkrT_d[:].rearrange("p (c e) -> p c e", c=NEC)
            )
            for r in range(1, 4):
                x_piece(r)
            wk2_s = persist.tile([P, NEC, P], BF16, tag="wk2")
            nc.sync.dma_start(wk2_s[:], wk2_d[:].rearrange("(c p) d -> p c d", p=P))
            wv2_s = persist.tile([P, NEC, P], BF16, tag="wv2")
            nc.sync.dma_start(wv2_s[:], wv2_d[:].rearrange("(c p) d -> p c d", p=P))
            # slow rot rows first (they gate the cheb chain); fast rows feed
            # the deferred M-finish ops
            rot_s = persist.tile([P, 4, N], BF16, tag="rot")
            nc.sync.dma_start(rot_s[:, 1, :], rot_d[P : 2 * P, :])
            nc.sync.dma_start(rot_s[:, 3, :], rot_d[3 * P : 4 * P, :])
            nc.sync.dma_start(rot_s[:, 0, :], rot_d[0:P, :])
            nc.sync.dma_start(rot_s[:, 2, :], rot_d[2 * P : 3 * P, :])
            sc_s = persist.tile([P, 2, NT], BF16, tag="sc")
            nc.sync.dma_start(sc_s[:], sc_d[:].rearrange("(k p) r -> p k r", p=P))
            SgF = persist.tile([P, NKT, 2, P], FP8, tag="SgF")
            nc.sync.dma_start(
                SgF[:], psiF_d[:].rearrange("p (t c j) -> p t c j", c=2, j=P)
            )
            # SgA free layout is chunk-major [c][t][j] so partition-sliced
            # chunk DMAs have 4KB contiguous runs. Only h0's tables load in
            # phase A; h1's are deferred past h0's attention start (the DMA
            # engines are a serialized resource on the startup critical path).
            SgA = []
            for h in range(HEADS_PER_CORE):
                t = persist.tile([P, 4, NKT, P], FP8, tag=f"SgA{h}", name=f"SgA{h}")
                SgA.append(t)

            def emit_sga_tables(h):
                tp = (1 - h) * D
                tps = slice(tp, tp + NT)
                nc.sync.dma_start(
                    SgA[h][tps, 0, :, :],
                    psiT_d[:, 0 : NKT * P].rearrange("p (t j) -> p t j", j=P),
                )
                nc.sync.dma_start(
                    SgA[h][tps, 2, :, :],
                    psiT_d[:, NKT * P :].rearrange("p (t j) -> p t j", j=P),
                )
                nc.sync.dma_start(
                    SgA[h][:, 3, :, :],
                    psiC_d[:].rearrange("p (t j) -> p t j", j=P),
                )
                # chunk1's T-half duplicates chunk0's (device-side dup)
                nc.scalar.dma_start(SgA[h][tps, 1, :, :], SgA[h][tps, 0, :, :])

            emit_sga_tables(0)
            wo_s = persist.tile([P, 2, E], BF16, tag="wo")
            nc.sync.dma_start(wo_s[:], wo2_d[:].rearrange("p (h e) -> p h e", h=2))
            # history lands last: its keys are first needed ~4 units into
            # attention, well after the rot/psi-gated startup chain
            for r in range(4):
                ks = slice(r * 512, (r + 1) * 512)
                nc.sync.dma_start(
                    axT_s[:, :, ks],
                    axT_d[:, ks].rearrange("(c p) k -> p c k", p=P),
                )

            identb = persist.tile([P, P], BF16, tag="identb")
            make_identity(nc, identb[:])

            # ---------------- persistent compute tiles ----------------
            # M chunks per head: 0=Uhi 1=Whi 2=[qhi|chi]
            # 3=[qhi-dup|clo] 4=[qlo|chi-dup] 5=[Wlo(f 0:64)|Ulo(f 64:128)]
            # (chunk 5 pairs with the psiA half-compensation stationary
            # [cos_hi(0:64)|sin_hi(64:128)] in the otherwise-wasted pad slot)
            M = []
            for h in range(HEADS_PER_CORE):
                m = persist.tile([P, 6, NQC, 512], FP8, tag=f"M{h}", name=f"M{h}")
                M.append(m)
            qv_s = persist.tile([P, N], BF16, tag="qv_s")
            vo = []
            for h in range(HEADS_PER_CORE):
                v = persist.tile([P, NKT, 66], BF16, tag=f"vo{h}", name=f"vo{h}")
                nc.gpsimd.memset(v[:, :, 64:66], 0.0)
                nc.gpsimd.memset(v[:, :, 64:65], 1.0)
                vo.append(v)
            # numTT: query-major pre-scaled numerators [q, s, d] (z separate);
            # numT: d-major via 128x128 transposes of s-tile PAIRS -- even
            # s-tile's d on partitions 0:64, odd on 64:128
            numT = []
            numTT = []
            zcs = []
            for h in range(HEADS_PER_CORE):
                t = persist.tile(
                    [P, NS // 2, P], BF16, tag=f"numT{h}", name=f"numT{h}"
                )
                numT.append(t)
                tt = persist.tile(
                    [P, NS, D], BF16, tag=f"numTT{h}", name=f"numTT{h}"
                )
                numTT.append(tt)
                zcs.append(
                    persist.tile([P, NS], F32, tag=f"zc{h}", name=f"zc{h}")
                )
            out_acc = persist.tile([P, NS, E], BF16, tag="out_acc")
            nbias = persist.tile([P, 1], F32, tag="nbias")
            nc.vector.memset(nbias[:], -SCORE_SHIFT)

            # ---------------- phase A: projections ----------------
            # q projection, both heads packed, emitted chunk-outer so the PE
            # starts as soon as each axT chunk lands. pq psums use the ph
            # banks (free until the av accumulators take them).
            pqs = [small_psum([P, 512], f"pq{qc}") for qc in range(NQC)]
            for qc in range(NQC):
                for c in range(NEC):
                    nc.tensor.matmul(
                        pqs[qc][:],
                        wq2_s[:, c, :],
                        axT[c][:, H + qc * 512 : H + (qc + 1) * 512],
                        start=(c == 0),
                        stop=(c == NEC - 1),
                    )
            for qc in range(NQC):
                pq = pqs[qc]
                qs = slice(qc * 512, (qc + 1) * 512)
                nc.vector.tensor_scalar_add(qv_s[:, qs], pq[:], vb_s[:])
                for h in range(HEADS_PER_CORE):
                    hp = slice(h * D, (h + 1) * D)
                    nc.vector.tensor_scalar_add(
                        M[h][hp, 2, qc, :], pq[hp, :], ub_s[hp]
                    )
                    nc.vector.scalar_tensor_tensor(
                        M[h][hp, 4, qc, :], pq[hp, :], ub_s[hp],
                        M[h][hp, 2, qc, :], ALU.add, ALU.subtract,
                    )

            def emit_uw_g_chunk(h, qc, j, sfd, ssd):
                # G: e 0:128 sin-fast + 256:384 cos-fast (sf);
                #    e 128:256 sin-slow + 384:512 cos-slow (ss)
                # one 1-bank psum + copy per chunk so at most one score-stream
                # slot is borrowed at a time
                hp = slice(h * D, (h + 1) * D)
                qs = slice(qc * 512, (qc + 1) * 512)
                half, jj = j // 2, j % 2
                dst = sfd if half == 0 else ssd
                g = pr.tile([P, 512], F32, tag="sp", name="g")
                nc.tensor.matmul(
                    g[:], wkr_s[hp, 2 * jj + half, :], qv_s[hp, qs],
                    start=True, stop=True,
                )
                nc.scalar.copy(dst[:, jj * 512 : (jj + 1) * 512], g[:])

            def emit_uw_g(h, qc, sfd, ssd):
                for j in range(4):
                    emit_uw_g_chunk(h, qc, j, sfd, ssd)

            def emit_uw_rot_slow(h, qc, ss, usw, me, add_eng=None):
                # slow half: rotate; compression happens in emit_uw_cheb
                add_eng = add_eng or nc.gpsimd
                qs = slice(qc * 512, (qc + 1) * 512)
                m5 = mst.tile([P, 512], BF16, tag="m1", name="m5")
                m6 = mst.tile([P, 512], BF16, tag="m2", name="m6")
                m7 = mst.tile([P, 512], BF16, tag="m3", name="m7")
                m8 = mst.tile([P, 512], BF16, tag="m4", name="m8")
                me[4].tensor_mul(m5[:], ss[:, 0:512], rot_s[:, 1, qs])
                me[5].tensor_mul(m6[:], ss[:, 512:1024], rot_s[:, 3, qs])
                me[6].tensor_mul(m7[:], ss[:, 512:1024], rot_s[:, 1, qs])
                me[7].tensor_mul(m8[:], ss[:, 0:512], rot_s[:, 3, qs])
                add_eng.tensor_add(usw[:, 0, :], m5[:], m6[:])
                add_eng.tensor_sub(usw[:, 1, :], m7[:], m8[:])

            def emit_uw_rot_fast(h, qc, sf, ubf, wbf, me, add_eng=None):
                # fast half: U = G*cos + Gc*sin ; W = Gc*cos - G*sin
                add_eng = add_eng or nc.gpsimd
                qs = slice(qc * 512, (qc + 1) * 512)
                m1 = mst.tile([P, 512], BF16, tag="m1")
                m2 = mst.tile([P, 512], BF16, tag="m2")
                m3 = mst.tile([P, 512], BF16, tag="m3")
                m4 = mst.tile([P, 512], BF16, tag="m4")
                me[0].tensor_mul(m1[:], sf[:, 0:512], rot_s[:, 0, qs])
                me[1].tensor_mul(m2[:], sf[:, 512:1024], rot_s[:, 2, qs])
                me[2].tensor_mul(m3[:], sf[:, 512:1024], rot_s[:, 0, qs])
                me[3].tensor_mul(m4[:], sf[:, 0:512], rot_s[:, 2, qs])
                add_eng.tensor_add(ubf[:], m1[:], m2[:])
                add_eng.tensor_sub(wbf[:], m3[:], m4[:])

            def emit_uw_rot_finish(h, qc, ubf, wbf, c0, c1, s5):
                c0(M[h][:, 0, qc, :], ubf[:])
                c1(M[h][:, 1, qc, :], wbf[:])
                # half lo-comp into the pad slot (partition-aligned halves)
                s5(M[h][0:D, 5, qc, :], wbf[0:D, :], M[h][0:D, 1, qc, :])
                s5(M[h][D:P, 5, qc, :], ubf[D:P, :], M[h][D:P, 0, qc, :])

            def emit_uw_rot(h, qc, sf, ss, usw):
                # combined form used for h1 during h0's attention: muls all
                # DVE (fast, 3/8 exps there), everything downstream of a mul
                # on Pool so the DVE queue never waits cross-engine.
                V, G = nc.vector, nc.gpsimd
                me = (V,) * 8
                ubf = mst.tile([P, 512], BF16, tag="ubf")
                wbf = mst.tile([P, 512], BF16, tag="wbf")
                emit_uw_rot_slow(h, qc, ss, usw, me)
                emit_uw_rot_fast(h, qc, sf, ubf, wbf, me)
                emit_uw_rot_finish(
                    h, qc, ubf, wbf, G.tensor_copy, G.tensor_copy, G.tensor_sub
                )

            def emit_uw_cheb(h, qc, usw, pc=None):
                # cheb coefs land on the head's opposite partition half
                po = (1 - h) * D
                cs = slice(po, po + NT)
                if pc is None:
                    pc = small_psum([P, 512], "pc")
                for k in range(2):
                    nc.tensor.matmul(
                        pc[cs, :], sc_s[:, k, :], usw[:, k, :],
                        start=(k == 0), stop=(k == 1),
                    )
                nc.scalar.copy(M[h][cs, 2, qc, :], pc[cs, :])
                nc.vector.tensor_sub(
                    M[h][cs, 3, qc, :], pc[cs, :], M[h][cs, 2, qc, :]
                )

            def emit_k(kc, pk=None):
                if pk is None:
                    pk = small_psum([P, 512], "pk")
                for c in range(NEC):
                    nc.tensor.matmul(
                        pk[:],
                        wk2_s[:, c, :],
                        axT[c][:, kc * 512 : (kc + 1) * 512],
                        start=(c == 0),
                        stop=(c == NEC - 1),
                    )
                ks = slice(4 * kc, 4 * kc + 4)
                for h in range(HEADS_PER_CORE):
                    hp = slice(h * D, (h + 1) * D)
                    pkv = pk[hp, :].rearrange("p (t j) -> p t j", j=P)
                    nc.scalar.copy(SgA[h][hp, 0, ks, :], pkv)
                    nc.vector.tensor_sub(
                        SgA[h][hp, 1, ks, :], pkv, SgA[h][hp, 0, ks, :]
                    )

            def emit_v(h, g, pv=None):
                hs = slice(h * D, (h + 1) * D)
                if pv is None:
                    pv = small_psum([P, 512], "pv")
                for k8 in range(8):
                    kt = g * 8 + k8
                    for c in range(NEC):
                        nc.tensor.matmul(
                            pv[:, k8 * D : (k8 + 1) * D],
                            axT[c][:, kt * P : (kt + 1) * P],
                            wv2_s[:, c, hs],
                            start=(c == 0),
                            stop=(c == NEC - 1),
                        )
                cp = nc.scalar.copy if (h + g) % 2 else nc.vector.tensor_copy
                cp(
                    vo[h][:, g * 8 : (g + 1) * 8, 0:D],
                    pv[:].rearrange("p (t d) -> p t d", d=D),
                )

            # h0 UW fully in phase A (streaming); h1's G copies land in a
            # persistent tile recycled from axT's tag so h1's rotation
            # (engine-only) can run during h0's attention.
            h1b = persist.tile(
                [P, NQC, 6, 512], BF16, tag="axT", name="h1buf"
            )
            h1buf = [h1b[:, u, :, :] for u in range(NQC)]

            # Phase A PE order: all G matmuls (only need q), then x-key
            # projections (their axT DMA lands early), then history keys,
            # then chebs (gated on the slow-rot chain) and finally the M
            # finish ops -- emitted last so the DVE queue never blocks the
            # attention exps behind a Pool dependency.
            # Phase A emission: q -> x-keys -> all G's -> x-values -> slow
            # rotations (DVE, matching the slow-first rot DMA) -> fast
            # rotations -> chebs -> finish. All same-queue chains; the only
            # cross-engine hops (ubf/wbf on Pool, M5 on Pool behind them)
            # are off the DVE queue so the attention exps aren't blocked.
            V, G, A = nc.vector, nc.gpsimd, nc.scalar
            me0 = (V,) * 8
            uwt = []
            for u in range(NQC):
                sf = gst.tile([P, 1024], BF16, tag="sf")
                ss = gst.tile([P, 1024], BF16, tag="ss")
                usw = gst.tile([P, 2, 512], BF16, tag="usw")
                ubf = gst.tile([P, 512], BF16, tag="ubf")
                wbf = gst.tile([P, 512], BF16, tag="wbf")
                uwt.append((sf, ss, usw, ubf, wbf))
            # x-key projections only in phase A; history keys are emitted
            # inside early h0 attention (their engine-queue work then sits
            # behind the first exps instead of gating them)
            for u in range(NQC):
                emit_k(4 + u)
            nc.scalar.dma_start(SgA[0][0:D, 2, 16:32, :], SgA[0][0:D, 0, 16:32, :])
            for u in range(NQC):
                emit_uw_g(0, u, uwt[u][0][:], uwt[u][1][:])
            for g in (2, 3):
                emit_v(0, g)
                emit_v(1, g)
            for u in range(NQC):
                emit_uw_rot_slow(0, u, uwt[u][1], uwt[u][2], me0, add_eng=V)
            for u in range(NQC):
                emit_uw_rot_fast(
                    0, u, uwt[u][0], uwt[u][3], uwt[u][4], me0, add_eng=V
                )
            # dups via DMA (off-engine): M chunk 3 q-half <- chunk 2 q-half;
            # chunk 4 cheb-half dups are per-qc so attention(qc0) only gates
            # on u=0's chain.
            nc.scalar.dma_start(M[0][0:D, 3, :, :], M[0][0:D, 2, :, :])
            cs0 = slice(D, D + NT)
            for u in range(NQC):
                emit_uw_cheb(0, u, uwt[u][2])
                nc.scalar.dma_start(
                    M[0][cs0, 4, u, :], M[0][cs0, 2, u, :]
                )
            for u in range(NQC):
                emit_uw_rot_finish(
                    0, u, uwt[u][3], uwt[u][4],
                    A.copy, A.copy, V.tensor_sub,
                )

            # ---------------- phase B: attention ----------------
            # Unit = one (key tile, query chunk): score psum is a 1-bank
            # [P, 512] tile from the 5-deep pr pool, so the
            # ps -> exp -> frees-slot chain never stalls the PE. exp
            # alternates Act (exact) / DVE (Schraudolph) per unit; during
            # h0's attention DVE also carries h1's rotation, so it only
            # takes 3 of 8 exps there.
            _expctr = [0]
            _dve_exp = {0: (1, 3, 5, 7), 1: (1, 3, 5, 7)}

            def emit_av(h, kt, kti, qc, pE, avv):
                for qt in range(4):
                    qg = qc * 4 + qt
                    bk, sl = divmod(qg, 6)
                    nc.tensor.matmul(
                        avv[bk][:, sl, :],
                        pE[:, qt * P : (qt + 1) * P],
                        vo[h][:, kt, 0:65],
                        start=(kti == 0 and qg in (0, 6, 12)),
                        stop=(kti == NKT - 1 and qg in (5, 11, 15)),
                        skip_group_check=True,
                    )

            def emit_unit(h, kt, kti, qc, avv, pend):
                ps = pr.tile([P, 512], F32, tag="sp", name="ps")
                nc.tensor.matmul(
                    ps[:], SgF[:, kt, :, :], M[h][:, 0:2, qc, :],
                    start=True, stop=False, perf_mode=DR,
                )
                nc.tensor.matmul(
                    ps[:], SgA[h][:, 0:2, kt, :], M[h][:, 2:4, qc, :],
                    start=False, stop=False, perf_mode=DR,
                )
                nc.tensor.matmul(
                    ps[:], SgA[h][:, 2:4, kt, :], M[h][:, 4:6, qc, :],
                    start=False, stop=True, perf_mode=DR,
                )
                if len(pend) >= 3:
                    emit_av(h, *pend.pop(0), avv)
                et = est.tile([P, 512], BF16, tag="E")
                if _expctr[0] % 8 not in _dve_exp[h]:
                    nc.scalar.activation(
                        et[:], ps[:], AF.Exp, scale=0.125, bias=nbias[:]
                    )
                else:
                    # Schraudolph: int16 bits = 128*(log2e*(s/8 - c) + 127)
                    nc.vector.tensor_scalar(
                        et[:].bitcast(I16), ps[:],
                        0.125 * P * LOG2E,
                        P * 127.0 - SCORE_SHIFT * P * LOG2E - 8.5,
                        ALU.mult, ALU.add,
                    )
                _expctr[0] += 1
                pend.append((kt, kti, qc, et))

            def emit_av_flush(h, avv, pend):
                while pend:
                    emit_av(h, *pend.pop(0), avv)

            zrecs = [
                persist.tile([P, NS], F32, tag=f"zrec{h}", name=f"zrec{h}")
                for h in range(HEADS_PER_CORE)
            ]

            def emit_z_scale(h, avv, s):
                # write numTT PRE-SCALED by 1/z (per-partition scalar per
                # s-tile) so the out-projection result needs no scaling
                bk, sl = divmod(s, 6)
                if s % 2 == 0:
                    nc.scalar.activation(
                        numTT[h][:, s, :], avv[bk][:, sl, 0:D], AF.Copy,
                        scale=zrecs[h][:, s : s + 1],
                    )
                else:
                    nc.vector.tensor_scalar_mul(
                        numTT[h][:, s, :], avv[bk][:, sl, 0:D],
                        zrecs[h][:, s : s + 1],
                    )

            def emit_z_qc(h, avv, qc):
                # one query chunk's denominators + pre-scaled numerators,
                # streamable as soon as that chunk's accumulation stops
                zc = zcs[h]
                s4 = slice(4 * qc, 4 * qc + 4)
                b0, l0 = divmod(4 * qc, 6)
                if l0 + 4 <= 6:
                    nc.vector.tensor_copy(zc[:, s4], avv[b0][:, l0 : l0 + 4, 64])
                else:
                    k = 6 - l0
                    nc.vector.tensor_copy(
                        zc[:, 4 * qc : 4 * qc + k], avv[b0][:, l0:6, 64]
                    )
                    nc.vector.tensor_copy(
                        zc[:, 4 * qc + k : 4 * qc + 4],
                        avv[b0 + 1][:, 0 : 4 - k, 64],
                    )
                nc.vector.reciprocal(zrecs[h][:, s4], zc[:, s4])
                for s in range(4 * qc, 4 * qc + 4):
                    emit_z_scale(h, avv, s)

            def emit_z_tr(h, s2):
                # transpose one PAIR of numerator s-tiles ([128,128] block)
                # back to d-major via the DMA xbar (off-engine)
                nc.sync.dma_start_transpose(
                    numT[h][:, s2, :],
                    numTT[h][:, 2 * s2 : 2 * s2 + 2, :],
                )

            def emit_z_tr_pe(h, s2, copy_eng):
                pz = pr.tile([P, P], BF16, tag="sp", name="pz")
                nc.tensor.transpose(
                    pz[:], numTT[h][:, 2 * s2 : 2 * s2 + 2, :], identb[:]
                )
                copy_eng(numT[h][:, s2, :], pz[:])

            def emit_out_s(h, s):
                # numT is pre-scaled by 1/z, so the psum->sbuf conversion is
                # a plain copy (alternating Act/DVE to spread the load)
                po = pr.tile([P, 512], F32, tag="sp", name="po")
                hp = (s % 2) * D
                nc.tensor.matmul(
                    po[:], numT[h][hp : hp + D, s // 2, :],
                    wo_s[hp : hp + D, h, :],
                    start=True, stop=True,
                )
                if s % 2 == 0:
                    nc.scalar.copy(out_acc[:, s, :], po[:])
                else:
                    nc.vector.tensor_copy(out_acc[:, s, :], po[:])
                if h == 0:
                    nc.sync.dma_start(
                        oA_d[:].rearrange("(s p) e -> p s e", p=P)[:, s, :],
                        out_acc[:, s, :],
                    )
                elif s in (3, 7, 11):
                    # h1 streams in 4-tile batches...
                    nc.sync.dma_start(
                        oB_d[:].rearrange("(s p) e -> p s e", p=P)[:, s - 3 : s + 1, :],
                        out_acc[:, s - 3 : s + 1, :],
                    )
                elif s in (13, 15):
                    # ...except the final quad goes as two pairs so the last
                    # transfer (the kernel's true tail) is half as long
                    nc.sync.dma_start(
                        oB_d[:].rearrange("(s p) e -> p s e", p=P)[:, s - 1 : s + 1, :],
                        out_acc[:, s - 1 : s + 1, :],
                    )

            # h0 attention with h1's G/rotation/cheb interleaved (their
            # elementwise runs on Pool/Act; DVE carries the exp stream)
            av0 = [
                ph.tile([P, 6 if j < 2 else 4, 65], F32, tag=f"bank{j}",
                        name=f"av0{j}")
                for j in range(3)
            ]
            def h0_interleave(gkt):
                # history-key projections moved inside attention: their
                # engine-queue work lands behind the first exps. Each kc's
                # chunk2 khi-dup follows its projection immediately; the
                # rotated kt order first touches kt0 at unit 16 (gkt 3).
                if 0 <= gkt <= 3:
                    kc = gkt
                    emit_k(kc, pk=pr.tile([P, 512], F32, tag="sp", name="pk1"))
                    ks = slice(4 * kc, 4 * kc + 4)
                    nc.scalar.dma_start(
                        SgA[0][0:D, 2, ks, :], SgA[0][0:D, 0, ks, :]
                    )
                if gkt == 2:
                    emit_v(0, 0, pv=pr.tile([P, 512], F32, tag="sp", name="pv1"))
                elif gkt == 3:
                    emit_v(0, 1, pv=pr.tile([P, 512], F32, tag="sp", name="pv1"))
                elif gkt == 4:
                    emit_v(1, 0, pv=pr.tile([P, 512], F32, tag="sp", name="pv1"))
                elif gkt == 5:
                    emit_v(1, 1, pv=pr.tile([P, 512], F32, tag="sp", name="pv1"))
                elif gkt == 6:
                    # h1's psi tables + dups, now that h0's attention flows
                    emit_sga_tables(1)
                    nc.scalar.dma_start(M[1][D:P, 3, :, :], M[1][D:P, 2, :, :])
                    nc.scalar.dma_start(SgA[1][D:P, 2, :, :], SgA[1][D:P, 0, :, :])
                # h1 prep: one G chunk per site, rotation after its 4 chunks,
                # cheb (one pr slot) once the Pool finishing ops drained
                elif 10 <= gkt <= 25:
                    u, j = divmod(gkt - 10, 4)
                    emit_uw_g_chunk(
                        1, u, j,
                        h1buf[u][:, 0:2, :].rearrange("p a b -> p (a b)"),
                        h1buf[u][:, 2:4, :].rearrange("p a b -> p (a b)"),
                    )
                if gkt in (15, 19, 23, 27):
                    u = (15, 19, 23, 27).index(gkt)
                    emit_uw_rot(
                        1, u,
                        h1buf[u][:, 0:2, :].rearrange("p a b -> p (a b)"),
                        h1buf[u][:, 2:4, :].rearrange("p a b -> p (a b)"),
                        h1buf[u][:, 4:6, :],
                    )
                if gkt in (18, 22, 26, 30):
                    u = (18, 22, 26, 30).index(gkt)
                    emit_uw_cheb(
                        1, u, h1buf[u][:, 4:6, :],
                        pc=pr.tile([P, 512], F32, tag="sp", name="pc1"),
                    )
                # h0's per-qc z chunks as each query chunk's accumulation ends
                if gkt in (9, 17, 25):
                    emit_z_qc(0, av0, (gkt - 9) // 8)

            pend0 = []
            _u0 = [0]
            for qc in range(NQC):
                for kti in range(NKT):
                    kt = (kti + NKT // 2) % NKT
                    emit_unit(0, kt, kti, qc, av0, pend0)
                    _u0[0] += 1
                    if _u0[0] % 4 == 0:
                        h0_interleave(_u0[0] // 4 - 1)

            emit_av_flush(0, av0, pend0)
            emit_z_qc(0, av0, 3)
            cs1 = slice(0, NT)
            nc.scalar.dma_start(M[1][cs1, 4, :, :], M[1][cs1, 2, :, :])

            # h1 attention with h0's transpose + output projection streamed
            # (out tile s at gkt = 6 + 3s//2, i.e. 2 tiles per 3 sites)
            _out_sched = {6 + (3 * s) // 2: s for s in range(NS)}
            av1 = [
                ph.tile([P, 6 if j < 2 else 4, 65], F32, tag=f"bank{j}",
                        name=f"av1{j}")
                for j in range(3)
            ]

            def h1_interleave(gkt):
                if 1 <= gkt <= 8:
                    emit_z_tr(0, gkt - 1)
                if gkt in _out_sched:
                    emit_out_s(0, _out_sched[gkt])
                # h1's own per-qc tail chunks stream during later qcs
                if gkt in (10, 18, 26):
                    c = (gkt - 10) // 8
                    emit_z_qc(1, av1, c)
                elif gkt in (11, 19, 27):
                    c = (gkt - 11) // 8
                    emit_z_tr_pe(1, 2 * c, nc.vector.tensor_copy)
                elif gkt in (12, 20, 28):
                    c = (gkt - 12) // 8
                    emit_out_s(1, 4 * c)
                    emit_out_s(1, 4 * c + 1)
                elif gkt in (13, 21, 29):
                    c = (gkt - 13) // 8
                    emit_z_tr_pe(1, 2 * c + 1, nc.scalar.copy)
                elif gkt in (14, 22, 30):
                    c = (gkt - 14) // 8
                    emit_out_s(1, 4 * c + 2)
                    emit_out_s(1, 4 * c + 3)

            pend1 = []
            _u1 = [0]
            for qc in range(NQC):
                for kti in range(NKT):
                    kt = (kti + NKT // 2) % NKT
                    emit_unit(1, kt, kti, qc, av1, pend1)
                    _u1[0] += 1
                    if _u1[0] % 4 == 0:
                        h1_interleave(_u1[0] // 4 - 1)
            # tail: only the last query chunk's drain remains
            emit_av_flush(1, av1, pend1)
            emit_z_qc(1, av1, 3)
            emit_z_tr_pe(1, 6, nc.vector.tensor_copy)
            emit_out_s(1, 12)
            emit_out_s(1, 13)
            emit_z_tr_pe(1, 7, nc.scalar.copy)
            emit_out_s(1, 14)
            emit_out_s(1, 15)

    nc.compile()
    return nc


_NC_CACHE = None


def _get_program():
    global _NC_CACHE
    if _NC_CACHE is None:
        _NC_CACHE = build_program()
    return _NC_CACHE


def _fp8_hl(x):
    hi = np.clip(np.asarray(x, np.float32), -240, 240).astype(ml_dtypes.float8_e4m3)
    lo = np.clip(
        np.asarray(x, np.float32) - hi.astype(np.float32), -240, 240
    ).astype(ml_dtypes.float8_e4m3)
    return hi, lo


def make_in_maps(x, history, w_q, w_k, w_v, w_kr, w_o, u_bias, v_bias):
    bf = ml_dtypes.bfloat16
    all_x = np.concatenate([history, x], axis=1)  # [B, HpN, E]

    inv_freq = 1.0 / (10000.0 ** (np.arange(0, E, 2, dtype=np.float64) / E))  # [256]
    ang_f = np.outer(inv_freq[:128], np.arange(HpN, dtype=np.float64) - H)
    xn = (np.arange(HpN, dtype=np.float64) - H) / 2048.0
    T = np.polynomial.chebyshev.chebvander(xn, NT - 1)  # [HpN, NT]
    ang_s = np.outer(xn * 2048.0, inv_freq[128:256])  # [HpN, 128]
    tgt = np.concatenate([np.sin(ang_s), np.cos(ang_s)], axis=1)  # [HpN, 256]
    coef, *_ = np.linalg.lstsq(T, tgt, rcond=None)  # [NT, 256]
    sc = np.ascontiguousarray(coef.T)  # [256, NT]: rows 0-127 sin, 128-255 cos

    sin_hi, _ = _fp8_hl(np.sin(ang_f))
    cos_hi, _ = _fp8_hl(np.cos(ang_f))
    T_hi, T_lo = _fp8_hl(T.T)  # [NT, HpN]
    sin_f = sin_hi.astype(np.float32)
    cos_f = cos_hi.astype(np.float32)
    # SgF partition-major: [p][t][c][j], chunks c = [sin_hi, cos_hi]
    psiF = np.ascontiguousarray(
        np.stack(
            [sin_f.reshape(P, NKT, P), cos_f.reshape(P, NKT, P)], axis=2
        ).reshape(P, NKT * 2 * P)
    )
    # shared cheb T basis [p(64)][hi/lo][t][j] (device places it per head)
    psiT = np.ascontiguousarray(
        np.stack(
            [
                T_hi.astype(np.float32).reshape(NT, NKT, P),
                T_lo.astype(np.float32).reshape(NT, NKT, P),
            ],
            axis=1,
        ).reshape(NT, 2 * NKT * P)
    )
    # fast-psi half-compensation stationary [cos_hi(f0:64)|sin_hi(f64:128)]:
    # pairs with M chunk 5 = [Wlo(f0:64)|Ulo(f64:128)]
    psiC = np.ascontiguousarray(
        np.concatenate([cos_f[0:D], sin_f[D:P]], axis=0).reshape(P, NKT * P)
    )

    ang_b = np.outer(inv_freq, np.arange(N, dtype=np.float64))  # [256, N]
    rot = np.ascontiguousarray(
        np.concatenate([np.cos(ang_b), np.sin(ang_b)]).astype(bf)
    )  # [512, N]: rows 0:128 cos-fast, 128:256 cos-slow, 256:384 sin-fast, ...

    clip8 = lambda a: np.clip(a, -240, 240).astype(ml_dtypes.float8_e4m3)

    in_maps = []
    for c in range(N_CORES):
        b = c // 4
        h0 = HEADS_PER_CORE * (c % 4)
        axT = np.ascontiguousarray(all_x[b].T).astype(bf)
        wq2 = np.concatenate([w_q[h0], w_q[h0 + 1]], axis=1).astype(bf)  # [E, 128]
        wk2 = np.concatenate([w_k[h0], w_k[h0 + 1]], axis=1).astype(bf)
        wv2 = np.concatenate([w_v[h0], w_v[h0 + 1]], axis=1).astype(bf)
        wkrT = np.concatenate(
            [w_kr[h0].T, w_kr[h0 + 1].T], axis=0
        ).astype(bf)  # [128, E]: rows 0:64 = head0 (d), 64:128 = head1
        wo1h = np.stack([w_o[h0], w_o[h0 + 1]], axis=1).reshape(D, 2 * E)
        wo2 = np.concatenate([wo1h, wo1h], axis=0).astype(bf)  # [P, 2E]
        in_maps.append(
            {
                "axT": axT,
                "rot": rot,
                "psiF": clip8(psiF),
                "psiT": clip8(psiT),
                "psiC": clip8(psiC),
                "sc": np.ascontiguousarray(sc).astype(bf),
                "wq2": np.ascontiguousarray(wq2),
                "wk2": np.ascontiguousarray(wk2),
                "wv2": np.ascontiguousarray(wv2),
                "wkrT": np.ascontiguousarray(wkrT),
                "wo2": np.ascontiguousarray(wo2),
                "ub2": np.ascontiguousarray(
                    np.concatenate([u_bias[h0], u_bias[h0 + 1]]).reshape(P, 1)
                ).astype(np.float32),
                "vb2": np.ascontiguousarray(
                    np.concatenate([v_bias[h0], v_bias[h0 + 1]]).reshape(P, 1)
                ).astype(np.float32),
            }
        )
    return in_maps


def run(inputs, trace=False, **kw):
    from concourse.bass_utils import run_bass_kernel_spmd

    nc = _get_program()
    in_maps = make_in_maps(
        np.asarray(inputs["x"], np.float32),
        np.asarray(inputs["history"], np.float32),
        np.asarray(inputs["w_q"], np.float32),
        np.asarray(inputs["w_k"], np.float32),
        np.asarray(inputs["w_v"], np.float32),
        np.asarray(inputs["w_kr"], np.float32),
        np.asarray(inputs["w_o"], np.float32),
        np.asarray(inputs["u_bias"], np.float32),
        np.asarray(inputs["v_bias"], np.float32),
    )
    res = run_bass_kernel_spmd(nc, in_maps, list(range(N_CORES)), trace=trace, **kw)
    out = np.zeros((B, N, E), np.float32)
    for c in range(N_CORES):
        out[c // 4] += res.results[c]["oA"].astype(np.float32).reshape(N, E)
        out[c // 4] += res.results[c]["oB"].astype(np.float32).reshape(N, E)
    return out, res


def kernel(**inputs):
    # mask is all ones (per the problem spec), so score masking is a no-op
    # and the tensor is ignored.
    out, _ = run(inputs, trace=False)
    return out



# revision 132
# speedup vs baseline: 1.0088x; 1.0088x over previous
"""Transformer-XL multi-head self-attention on 8 Trainium2 NeuronCores.

Sharding: core c handles batch b = c//4 and heads {2*(c%4), 2*(c%4)+1}
(data-parallel over B x tensor-parallel over heads). Each core produces a
partial [N, E] output (its heads' w_o contributions); the host sums the 4
partials per batch element.

The XL relative-position term BD[i,j] = (q_i+v)·BDk[j-i+N-1] is computed
without the rel_shift gather via per-query rotation (angle-difference
identities): BD^T = Psi @ UW with Psi a shape-derived constant basis
(128 exact sin rows + 128 exact cos rows + 64 Chebyshev rows for the slow
frequencies) and UW per-query rotated coefficients.

Scores run on the PE in fp8e4 DoubleRow mode (0.5 cycles/row in the cost
model) with hi/lo error compensation: a bf16-accurate operand x is split
as x = hi + lo with hi = fp8(x), lo = fp8(x - hi), keeping selected cross
terms. Per 128-key tile the contraction is 6 chunks of 128 rows consumed
by 3 DoubleRow calls:
  [sin|cos]x[Uhi|Whi],
  [khi|Thi]x[qhi|chi], [klo|Thi]x[qhi|clo], [khi|Tlo]x[qlo|chi], pad
where T/c are the Chebyshev basis/coefficients and k/q carry the content
term (q+u)·k. The U/W (fast psi coefficient) lo-compensation is dropped
(one-sided both psi and U/W): host-side simulation puts the end-to-end
max-rel error at ~1.4% vs the 2% gate (vs ~1.2% with the compensation).
The value path (exp, V, attn@V, output projection) stays in bf16: fp8
noise there does not average out. exp is spread over Act/DVE/Pool
(Schraudolph on DVE/Pool; the extra Schraudolph noise is ~free: ~1.47%
even if every tile uses it).
"""

import sys

sys.path.insert(0, "/opt/trn_rl_repo")

import ml_dtypes
import numpy as np

import concourse.bass as bass
import concourse.mybir as mybir
from concourse import bacc
from concourse.masks import make_identity
from concourse.tile import TileContext

F32 = mybir.dt.float32
BF16 = mybir.dt.bfloat16
FP8 = mybir.dt.float8e4
I16 = mybir.dt.int16
AF = mybir.ActivationFunctionType
ALU = mybir.AluOpType
DR = mybir.MatmulPerfMode.DoubleRow

B, N, H, E, NH, D = 2, 2048, 2048, 512, 8, 64
HpN = H + N  # 4096
P = 128
NKT = HpN // P  # 32 key tiles
NPAIR = NKT // 2  # 16 key-tile pairs
NQC = N // 512  # 4 query chunks of 512
NEC = E // P  # 4 contraction chunks over E
NS = N // P  # 16 output row tiles
NT = 64  # chebyshev terms
HEADS_PER_CORE = 2
N_CORES = 8

LOG2E = 1.4426950408889634
SCORE_SHIFT = 1.5  # exp(s - c): cancels in softmax, bounds exp values
# exp tile engine rotation: (ctr % MOD) -> r < EXP_ACT on Act (exact),
# rest on DVE (Schraudolph). GPSIMD cannot read PSUM so Pool is out.
# Strict alternation: consecutive units' exps overlap across the two
# engines (each engine sees one ~1.1us exp per two 858ns PE units).
EXP_MOD, EXP_ACT = 2, 1


def build_program():
    nc = bacc.Bacc("TRN2", target_bir_lowering=False, debug=False)

    axT_d = nc.declare_dram_parameter("axT", [E, HpN], BF16, isOutput=False)
    rot_d = nc.declare_dram_parameter("rot", [E, N], BF16, isOutput=False)
    # SgF: shared fast-psi chunks, partition-major [p][t][c][j] so the DMA is
    # an identity layout with 8KB per-partition runs
    psiF_d = nc.declare_dram_parameter("psiF", [P, NKT * 2 * P], FP8, isOutput=False)
    # shared cheb T basis rows [p(64)][hi/lo][t][j]; identical for both heads
    # (placed at opposite partition halves on device)
    psiT_d = nc.declare_dram_parameter("psiT", [NT, 2 * NKT * P], FP8, isOutput=False)
    # fast-psi half-compensation stationary [cos_hi(f0:64)|sin_hi(f64:128)],
    # shared by both heads: [p][t][j]
    psiC_d = nc.declare_dram_parameter("psiC", [P, NKT * P], FP8, isOutput=False)
    sc_d = nc.declare_dram_parameter("sc", [2 * P, NT], BF16, isOutput=False)
    wq2_d = nc.declare_dram_parameter("wq2", [E, P], BF16, isOutput=False)
    wk2_d = nc.declare_dram_parameter("wk2", [E, P], BF16, isOutput=False)
    wv2_d = nc.declare_dram_parameter("wv2", [E, P], BF16, isOutput=False)
    wkrT_d = nc.declare_dram_parameter("wkrT", [P, E], BF16, isOutput=False)
    # wo duplicated on both partition halves (odd numT s-tiles live at 64:128)
    wo2_d = nc.declare_dram_parameter("wo2", [P, 2 * E], BF16, isOutput=False)
    ub2_d = nc.declare_dram_parameter("ub2", [P, 1], F32, isOutput=False)
    vb2_d = nc.declare_dram_parameter("vb2", [P, 1], F32, isOutput=False)
    # two per-head partial outputs (host sums): h0 streams during h1's
    # attention; h1 drains at the tail
    oA_d = nc.declare_dram_parameter("oA", [N, E], BF16, isOutput=True)
    oB_d = nc.declare_dram_parameter("oB", [N, E], BF16, isOutput=True)

    with TileContext(nc) as tc:
        with (
            tc.tile_pool(name="persist", bufs=1) as persist,
            tc.tile_pool(name="gst", bufs=4) as gst,       # G copies stream
            tc.tile_pool(name="mst", bufs=2) as mst,       # rotation temps
            tc.tile_pool(name="est", bufs=8) as est,       # exp tiles
            tc.tile_pool(name="dram", bufs=1, space="DRAM") as dram_pool,
            tc.tile_pool(name="pr", bufs=5, space="PSUM") as pr,   # 5x [P,512]
            tc.tile_pool(name="ph", bufs=1, space="PSUM") as ph,   # 3x [P,512]
        ):
            _sm = [0]

            def small_psum(shape, name, dtype=F32, tag=None):
                if tag is None:
                    i = _sm[0] % 3
                    _sm[0] += 1
                    tag = f"bank{i}"
                return ph.tile(shape, dtype, tag=tag, name=name)

            # ---------------- DMAs ----------------
            # One prioritized stream on the sync queue: the DMA engines are a
            # serialized resource, so emission order here IS the priority.
            # q proj needs {wq2, x-half}; the uw chain adds {wkr, rot, sc};
            # emit_k(4..7)/emit_v(x) add {wk2, wv2}; history keys come next,
            # then the attention-only psi tables and wo.
            wq2_s = persist.tile([P, NEC, P], BF16, tag="wq2")
            nc.sync.dma_start(wq2_s[:], wq2_d[:].rearrange("(c p) d -> p c d", p=P))
            # first x piece immediately after wq2 -- the small-weight DMAs'
            # per-transfer HWDGE overheads would delay the very first matmul
            axT_s = persist.tile([P, NEC, HpN], BF16, tag="axT", name="axT")
            axT = [axT_s[:, c, :] for c in range(NEC)]

            def x_piece(r):
                ks = slice(H + r * 512, H + (r + 1) * 512)
                nc.sync.dma_start(
                    axT_s[:, :, ks],
                    axT_d[:, ks].rearrange("(c p) k -> p c k", p=P),
                )

            x_piece(0)
            ub_s = persist.tile([P, 1], F32, tag="ub")
            nc.sync.dma_start(ub_s[:], ub2_d[:])
            vb_s = persist.tile([P, 1], F32, tag="vb")
            nc.sync.dma_start(vb_s[:], vb2_d[:])
            # wkr stacked on partitions: rows 0:64 = head0 d, 64:128 = head1 d
            wkr_s = persist.tile([P, NEC, P], BF16, tag="wkr")
            nc.sync.dma_start(
                wkr_s[:], wkrT_d[:].rearrange("p (c e) -> p c e", c=NEC)
            )
            for r in range(1, 4):
                x_piece(r)
            wk2_s = persist.tile([P, NEC, P], BF16, tag="wk2")
            nc.sync.dma_start(wk2_s[:], wk2_d[:].rearrange("(c p) d -> p c d", p=P))
            wv2_s = persist.tile([P, NEC, P], BF16, tag="wv2")
            nc.sync.dma_start(wv2_s[:], wv2_d[:].rearrange("(c p) d -> p c d", p=P))
            # slow rot rows first (they gate the cheb chain); fast rows feed
            # the deferred M-finish ops
            rot_s = persist.tile([P, 4, N], BF16, tag="rot")
            nc.sync.dma_start(rot_s[:, 1, :], rot_d[P : 2 * P, :])
            nc.sync.dma_start(rot_s[:, 3, :], rot_d[3 * P : 4 * P, :])
            nc.sync.dma_start(rot_s[:, 0, :], rot_d[0:P, :])
            nc.sync.dma_start(rot_s[:, 2, :], rot_d[2 * P : 3 * P, :])
            sc_s = persist.tile([P, 2, NT], BF16, tag="sc")
            nc.sync.dma_start(sc_s[:], sc_d[:].rearrange("(k p) r -> p k r", p=P))
            SgF = persist.tile([P, NKT, 2, P], FP8, tag="SgF")
            nc.sync.dma_start(
                SgF[:], psiF_d[:].rearrange("p (t c j) -> p t c j", c=2, j=P)
            )
            # SgA free layout is chunk-major [c][t][j] so partition-sliced
            # chunk DMAs have 4KB contiguous runs. Only h0's tables load in
            # phase A; h1's are deferred past h0's attention start (the DMA
            # engines are a serialized resource on the startup critical path).
            SgA = []
            for h in range(HEADS_PER_CORE):
                t = persist.tile([P, 4, NKT, P], FP8, tag=f"SgA{h}", name=f"SgA{h}")
                SgA.append(t)

            def emit_sga_tables(h):
                tp = (1 - h) * D
                tps = slice(tp, tp + NT)
                nc.sync.dma_start(
                    SgA[h][tps, 0, :, :],
                    psiT_d[:, 0 : NKT * P].rearrange("p (t j) -> p t j", j=P),
                )
                nc.sync.dma_start(
                    SgA[h][tps, 2, :, :],
                    psiT_d[:, NKT * P :].rearrange("p (t j) -> p t j", j=P),
                )
                nc.sync.dma_start(
                    SgA[h][:, 3, :, :],
                    psiC_d[:].rearrange("p (t j) -> p t j", j=P),
                )
                # chunk1's T-half duplicates chunk0's (device-side dup)
                nc.scalar.dma_start(SgA[h][tps, 1, :, :], SgA[h][tps, 0, :, :])

            emit_sga_tables(0)
            wo_s = persist.tile([P, 2, E], BF16, tag="wo")
            nc.sync.dma_start(wo_s[:], wo2_d[:].rearrange("p (h e) -> p h e", h=2))
            # history lands last: its keys are first needed ~4 units into
            # attention, well after the rot/psi-gated startup chain
            for r in range(4):
                ks = slice(r * 512, (r + 1) * 512)
                nc.sync.dma_start(
                    axT_s[:, :, ks],
                    axT_d[:, ks].rearrange("(c p) k -> p c k", p=P),
                )

            identb = persist.tile([P, P], BF16, tag="identb")
            make_identity(nc, identb[:])

            # ---------------- persistent compute tiles ----------------
            # M chunks per head: 0=Uhi 1=Whi 2=[qhi|chi]
            # 3=[qhi-dup|clo] 4=[qlo|chi-dup] 5=[Wlo(f 0:64)|Ulo(f 64:128)]
            # (chunk 5 pairs with the psiA half-compensation stationary
            # [cos_hi(0:64)|sin_hi(64:128)] in the otherwise-wasted pad slot)
            M = []
            for h in range(HEADS_PER_CORE):
                m = persist.tile([P, 6, NQC, 512], FP8, tag=f"M{h}", name=f"M{h}")
                M.append(m)
            qv_s = persist.tile([P, N], BF16, tag="qv_s")
            vo = []
            for h in range(HEADS_PER_CORE):
                v = persist.tile([P, NKT, 66], BF16, tag=f"vo{h}", name=f"vo{h}")
                nc.gpsimd.memset(v[:, :, 64:66], 0.0)
                nc.gpsimd.memset(v[:, :, 64:65], 1.0)
                vo.append(v)
            # numTT: query-major pre-scaled numerators [q, s, d] (z separate);
            # numT: d-major via 128x128 transposes of s-tile PAIRS -- even
            # s-tile's d on partitions 0:64, odd on 64:128
            numT = []
            numTT = []
            zcs = []
            for h in range(HEADS_PER_CORE):
                t = persist.tile(
                    [P, NS // 2, P], BF16, tag=f"numT{h}", name=f"numT{h}"
                )
                numT.append(t)
                tt = persist.tile(
                    [P, NS, D], BF16, tag=f"numTT{h}", name=f"numTT{h}"
                )
                numTT.append(tt)
                zcs.append(
                    persist.tile([P, NS], F32, tag=f"zc{h}", name=f"zc{h}")
                )
            out_acc = persist.tile([P, NS, E], BF16, tag="out_acc")
            nbias = persist.tile([P, 1], F32, tag="nbias")
            nc.vector.memset(nbias[:], -SCORE_SHIFT)

            # ---------------- phase A: projections ----------------
            # q projection, both heads packed, emitted chunk-outer so the PE
            # starts as soon as each axT chunk lands. pq psums use the ph
            # banks (free until the av accumulators take them).
            pqs = [small_psum([P, 512], f"pq{qc}") for qc in range(NQC)]
            for qc in range(NQC):
                for c in range(NEC):
                    nc.tensor.matmul(
                        pqs[qc][:],
                        wq2_s[:, c, :],
                        axT[c][:, H + qc * 512 : H + (qc + 1) * 512],
                        start=(c == 0),
                        stop=(c == NEC - 1),
                    )
            for qc in range(NQC):
                pq = pqs[qc]
                qs = slice(qc * 512, (qc + 1) * 512)
                nc.vector.tensor_scalar_add(qv_s[:, qs], pq[:], vb_s[:])
                for h in range(HEADS_PER_CORE):
                    hp = slice(h * D, (h + 1) * D)
                    nc.vector.tensor_scalar_add(
                        M[h][hp, 2, qc, :], pq[hp, :], ub_s[hp]
                    )
                    nc.vector.scalar_tensor_tensor(
                        M[h][hp, 4, qc, :], pq[hp, :], ub_s[hp],
                        M[h][hp, 2, qc, :], ALU.add, ALU.subtract,
                    )

            def emit_uw_g_chunk(h, qc, j, sfd, ssd):
                # G: e 0:128 sin-fast + 256:384 cos-fast (sf);
                #    e 128:256 sin-slow + 384:512 cos-slow (ss)
                # one 1-bank psum + copy per chunk so at most one score-stream
                # slot is borrowed at a time
                hp = slice(h * D, (h + 1) * D)
                qs = slice(qc * 512, (qc + 1) * 512)
                half, jj = j // 2, j % 2
                dst = sfd if half == 0 else ssd
                g = pr.tile([P, 512], F32, tag="sp", name="g")
                nc.tensor.matmul(
                    g[:], wkr_s[hp, 2 * jj + half, :], qv_s[hp, qs],
                    start=True, stop=True,
                )
                nc.scalar.copy(dst[:, jj * 512 : (jj + 1) * 512], g[:])

            def emit_uw_g(h, qc, sfd, ssd):
                for j in range(4):
                    emit_uw_g_chunk(h, qc, j, sfd, ssd)

            def emit_uw_rot_slow(h, qc, ss, usw, me, add_eng=None):
                # slow half: rotate; compression happens in emit_uw_cheb
                add_eng = add_eng or nc.gpsimd
                qs = slice(qc * 512, (qc + 1) * 512)
                m5 = mst.tile([P, 512], BF16, tag="m1", name="m5")
                m6 = mst.tile([P, 512], BF16, tag="m2", name="m6")
                m7 = mst.tile([P, 512], BF16, tag="m3", name="m7")
                m8 = mst.tile([P, 512], BF16, tag="m4", name="m8")
                me[4].tensor_mul(m5[:], ss[:, 0:512], rot_s[:, 1, qs])
                me[5].tensor_mul(m6[:], ss[:, 512:1024], rot_s[:, 3, qs])
                me[6].tensor_mul(m7[:], ss[:, 512:1024], rot_s[:, 1, qs])
                me[7].tensor_mul(m8[:], ss[:, 0:512], rot_s[:, 3, qs])
                add_eng.tensor_add(usw[:, 0, :], m5[:], m6[:])
                add_eng.tensor_sub(usw[:, 1, :], m7[:], m8[:])

            def emit_uw_rot_fast(h, qc, sf, ubf, wbf, me, add_eng=None):
                # fast half: U = G*cos + Gc*sin ; W = Gc*cos - G*sin
                add_eng = add_eng or nc.gpsimd
                qs = slice(qc * 512, (qc + 1) * 512)
                m1 = mst.tile([P, 512], BF16, tag="m1")
                m2 = mst.tile([P, 512], BF16, tag="m2")
                m3 = mst.tile([P, 512], BF16, tag="m3")
                m4 = mst.tile([P, 512], BF16, tag="m4")
                me[0].tensor_mul(m1[:], sf[:, 0:512], rot_s[:, 0, qs])
                me[1].tensor_mul(m2[:], sf[:, 512:1024], rot_s[:, 2, qs])
                me[2].tensor_mul(m3[:], sf[:, 512:1024], rot_s[:, 0, qs])
                me[3].tensor_mul(m4[:], sf[:, 0:512], rot_s[:, 2, qs])
                add_eng.tensor_add(ubf[:], m1[:], m2[:])
                add_eng.tensor_sub(wbf[:], m3[:], m4[:])

            def emit_uw_rot_finish(h, qc, ubf, wbf, c0, c1, s5):
                c0(M[h][:, 0, qc, :], ubf[:])
                c1(M[h][:, 1, qc, :], wbf[:])
                # half lo-comp into the pad slot (partition-aligned halves)
                s5(M[h][0:D, 5, qc, :], wbf[0:D, :], M[h][0:D, 1, qc, :])
                s5(M[h][D:P, 5, qc, :], ubf[D:P, :], M[h][D:P, 0, qc, :])

            def emit_uw_rot(h, qc, sf, ss, usw):
                # combined form used for h1 during h0's attention: muls all
                # DVE (fast, 3/8 exps there), everything downstream of a mul
                # on Pool so the DVE queue never waits cross-engine.
                V, G = nc.vector, nc.gpsimd
                me = (V,) * 8
                ubf = mst.tile([P, 512], BF16, tag="ubf")
                wbf = mst.tile([P, 512], BF16, tag="wbf")
                emit_uw_rot_slow(h, qc, ss, usw, me)
                emit_uw_rot_fast(h, qc, sf, ubf, wbf, me)
                emit_uw_rot_finish(
                    h, qc, ubf, wbf, G.tensor_copy, G.tensor_copy, G.tensor_sub
                )

            def emit_uw_cheb(h, qc, usw, pc=None):
                # cheb coefs land on the head's opposite partition half
                po = (1 - h) * D
                cs = slice(po, po + NT)
                if pc is None:
                    pc = small_psum([P, 512], "pc")
                for k in range(2):
                    nc.tensor.matmul(
                        pc[cs, :], sc_s[:, k, :], usw[:, k, :],
                        start=(k == 0), stop=(k == 1),
                    )
                nc.scalar.copy(M[h][cs, 2, qc, :], pc[cs, :])
                nc.vector.tensor_sub(
                    M[h][cs, 3, qc, :], pc[cs, :], M[h][cs, 2, qc, :]
                )

            def emit_k(kc, pk=None):
                if pk is None:
                    pk = small_psum([P, 512], "pk")
                for c in range(NEC):
                    nc.tensor.matmul(
                        pk[:],
                        wk2_s[:, c, :],
                        axT[c][:, kc * 512 : (kc + 1) * 512],
                        start=(c == 0),
                        stop=(c == NEC - 1),
                    )
                ks = slice(4 * kc, 4 * kc + 4)
                for h in range(HEADS_PER_CORE):
                    hp = slice(h * D, (h + 1) * D)
                    pkv = pk[hp, :].rearrange("p (t j) -> p t j", j=P)
                    nc.scalar.copy(SgA[h][hp, 0, ks, :], pkv)
                    nc.vector.tensor_sub(
                        SgA[h][hp, 1, ks, :], pkv, SgA[h][hp, 0, ks, :]
                    )

            def emit_v(h, g, pv=None):
                hs = slice(h * D, (h + 1) * D)
                if pv is None:
                    pv = small_psum([P, 512], "pv")
                for k8 in range(8):
                    kt = g * 8 + k8
                    for c in range(NEC):
                        nc.tensor.matmul(
                            pv[:, k8 * D : (k8 + 1) * D],
                            axT[c][:, kt * P : (kt + 1) * P],
                            wv2_s[:, c, hs],
                            start=(c == 0),
                            stop=(c == NEC - 1),
                        )
                cp = nc.scalar.copy if (h + g) % 2 else nc.vector.tensor_copy
                cp(
                    vo[h][:, g * 8 : (g + 1) * 8, 0:D],
                    pv[:].rearrange("p (t d) -> p t d", d=D),
                )

            # h0 UW fully in phase A (streaming); h1's G copies land in a
            # persistent tile recycled from axT's tag so h1's rotation
            # (engine-only) can run during h0's attention.
            h1b = persist.tile(
                [P, NQC, 6, 512], BF16, tag="axT", name="h1buf"
            )
            h1buf = [h1b[:, u, :, :] for u in range(NQC)]

            # Phase A PE order: all G matmuls (only need q), then x-key
            # projections (their axT DMA lands early), then history keys,
            # then chebs (gated on the slow-rot chain) and finally the M
            # finish ops -- emitted last so the DVE queue never blocks the
            # attention exps behind a Pool dependency.
            # Phase A emission: q -> x-keys -> all G's -> x-values -> slow
            # rotations (DVE, matching the slow-first rot DMA) -> fast
            # rotations -> chebs -> finish. All same-queue chains; the only
            # cross-engine hops (ubf/wbf on Pool, M5 on Pool behind them)
            # are off the DVE queue so the attention exps aren't blocked.
            V, G, A = nc.vector, nc.gpsimd, nc.scalar
            me0 = (V,) * 8
            uwt = []
            for u in range(NQC):
                sf = gst.tile([P, 1024], BF16, tag="sf")
                ss = gst.tile([P, 1024], BF16, tag="ss")
                usw = gst.tile([P, 2, 512], BF16, tag="usw")
                ubf = gst.tile([P, 512], BF16, tag="ubf")
                wbf = gst.tile([P, 512], BF16, tag="wbf")
                uwt.append((sf, ss, usw, ubf, wbf))
            # x-key projections only in phase A; history keys are emitted
            # inside early h0 attention (their engine-queue work then sits
            # behind the first exps instead of gating them)
            for u in range(NQC):
                emit_k(4 + u)
            nc.scalar.dma_start(SgA[0][0:D, 2, 16:32, :], SgA[0][0:D, 0, 16:32, :])
            for u in range(NQC):
                emit_uw_g(0, u, uwt[u][0][:], uwt[u][1][:])
            for g in (2, 3):
                emit_v(0, g)
                emit_v(1, g)
            for u in range(NQC):
                emit_uw_rot_slow(0, u, uwt[u][1], uwt[u][2], me0, add_eng=V)
            for u in range(NQC):
                emit_uw_rot_fast(0, u, uwt[u][0], uwt[u][3], uwt[u][4], me0)
            # dups via DMA (off-engine): M chunk 3 q-half <- chunk 2 q-half;
            # chunk 4 cheb-half dups are per-qc so attention(qc0) only gates
            # on u=0's chain.
            nc.scalar.dma_start(M[0][0:D, 3, :, :], M[0][0:D, 2, :, :])
            cs0 = slice(D, D + NT)
            for u in range(NQC):
                emit_uw_cheb(0, u, uwt[u][2])
                nc.scalar.dma_start(
                    M[0][cs0, 4, u, :], M[0][cs0, 2, u, :]
                )
            for u in range(NQC):
                emit_uw_rot_finish(
                    0, u, uwt[u][3], uwt[u][4],
                    A.copy, A.copy, V.tensor_sub,
                )

            # ---------------- phase B: attention ----------------
            # Unit = one (key tile, query chunk): score psum is a 1-bank
            # [P, 512] tile from the 5-deep pr pool, so the
            # ps -> exp -> frees-slot chain never stalls the PE. exp
            # alternates Act (exact) / DVE (Schraudolph) per unit; during
            # h0's attention DVE also carries h1's rotation, so it only
            # takes 3 of 8 exps there.
            _expctr = [0]
            _dve_exp = {0: (1, 3, 5, 7), 1: (1, 3, 5, 7)}

            def emit_av(h, kt, kti, qc, pE, avv):
                for qt in range(4):
                    qg = qc * 4 + qt
                    bk, sl = divmod(qg, 6)
                    nc.tensor.matmul(
                        avv[bk][:, sl, :],
                        pE[:, qt * P : (qt + 1) * P],
                        vo[h][:, kt, 0:65],
                        start=(kti == 0 and qg in (0, 6, 12)),
                        stop=(kti == NKT - 1 and qg in (5, 11, 15)),
                        skip_group_check=True,
                    )

            def emit_unit(h, kt, kti, qc, avv, pend):
                ps = pr.tile([P, 512], F32, tag="sp", name="ps")
                nc.tensor.matmul(
                    ps[:], SgF[:, kt, :, :], M[h][:, 0:2, qc, :],
                    start=True, stop=False, perf_mode=DR,
                )
                nc.tensor.matmul(
                    ps[:], SgA[h][:, 0:2, kt, :], M[h][:, 2:4, qc, :],
                    start=False, stop=False, perf_mode=DR,
                )
                nc.tensor.matmul(
                    ps[:], SgA[h][:, 2:4, kt, :], M[h][:, 4:6, qc, :],
                    start=False, stop=True, perf_mode=DR,
                )
                if len(pend) >= 3:
                    emit_av(h, *pend.pop(0), avv)
                et = est.tile([P, 512], BF16, tag="E")
                if _expctr[0] % 8 not in _dve_exp[h]:
                    nc.scalar.activation(
                        et[:], ps[:], AF.Exp, scale=0.125, bias=nbias[:]
                    )
                else:
                    # Schraudolph: int16 bits = 128*(log2e*(s/8 - c) + 127)
                    nc.vector.tensor_scalar(
                        et[:].bitcast(I16), ps[:],
                        0.125 * P * LOG2E,
                        P * 127.0 - SCORE_SHIFT * P * LOG2E - 8.5,
                        ALU.mult, ALU.add,
                    )
                _expctr[0] += 1
                pend.append((kt, kti, qc, et))

            def emit_av_flush(h, avv, pend):
                while pend:
                    emit_av(h, *pend.pop(0), avv)

            zrecs = [
                persist.tile([P, NS], F32, tag=f"zrec{h}", name=f"zrec{h}")
                for h in range(HEADS_PER_CORE)
            ]

            def emit_z_scale(h, avv, s):
                # write numTT PRE-SCALED by 1/z (per-partition scalar per
                # s-tile) so the out-projection result needs no scaling
                bk, sl = divmod(s, 6)
                if s % 2 == 0:
                    nc.scalar.activation(
                        numTT[h][:, s, :], avv[bk][:, sl, 0:D], AF.Copy,
                        scale=zrecs[h][:, s : s + 1],
                    )
                else:
                    nc.vector.tensor_scalar_mul(
                        numTT[h][:, s, :], avv[bk][:, sl, 0:D],
                        zrecs[h][:, s : s + 1],
                    )

            def emit_z_qc(h, avv, qc):
                # one query chunk's denominators + pre-scaled numerators,
                # streamable as soon as that chunk's accumulation stops
                zc = zcs[h]
                s4 = slice(4 * qc, 4 * qc + 4)
                b0, l0 = divmod(4 * qc, 6)
                if l0 + 4 <= 6:
                    nc.vector.tensor_copy(zc[:, s4], avv[b0][:, l0 : l0 + 4, 64])
                else:
                    k = 6 - l0
                    nc.vector.tensor_copy(
                        zc[:, 4 * qc : 4 * qc + k], avv[b0][:, l0:6, 64]
                    )
                    nc.vector.tensor_copy(
                        zc[:, 4 * qc + k : 4 * qc + 4],
                        avv[b0 + 1][:, 0 : 4 - k, 64],
                    )
                nc.vector.reciprocal(zrecs[h][:, s4], zc[:, s4])
                for s in range(4 * qc, 4 * qc + 4):
                    emit_z_scale(h, avv, s)

            def emit_z_tr(h, s2):
                # transpose one PAIR of numerator s-tiles ([128,128] block)
                # back to d-major via the DMA xbar (off-engine)
                nc.sync.dma_start_transpose(
                    numT[h][:, s2, :],
                    numTT[h][:, 2 * s2 : 2 * s2 + 2, :],
                )

            def emit_z_tr_pe(h, s2, copy_eng):
                pz = pr.tile([P, P], BF16, tag="sp", name="pz")
                nc.tensor.transpose(
                    pz[:], numTT[h][:, 2 * s2 : 2 * s2 + 2, :], identb[:]
                )
                copy_eng(numT[h][:, s2, :], pz[:])

            def emit_out_s(h, s):
                # numT is pre-scaled by 1/z, so the psum->sbuf conversion is
                # a plain copy (alternating Act/DVE to spread the load)
                po = pr.tile([P, 512], F32, tag="sp", name="po")
                hp = (s % 2) * D
                nc.tensor.matmul(
                    po[:], numT[h][hp : hp + D, s // 2, :],
                    wo_s[hp : hp + D, h, :],
                    start=True, stop=True,
                )
                if s % 2 == 0:
                    nc.scalar.copy(out_acc[:, s, :], po[:])
                else:
                    nc.vector.tensor_copy(out_acc[:, s, :], po[:])
                if h == 0:
                    nc.sync.dma_start(
                        oA_d[:].rearrange("(s p) e -> p s e", p=P)[:, s, :],
                        out_acc[:, s, :],
                    )
                elif s in (3, 7, 11):
                    # h1 streams in 4-tile batches...
                    nc.sync.dma_start(
                        oB_d[:].rearrange("(s p) e -> p s e", p=P)[:, s - 3 : s + 1, :],
                        out_acc[:, s - 3 : s + 1, :],
                    )
                elif s in (13, 15):
                    # ...except the final quad goes as two pairs so the last
                    # transfer (the kernel's true tail) is half as long
                    nc.sync.dma_start(
                        oB_d[:].rearrange("(s p) e -> p s e", p=P)[:, s - 1 : s + 1, :],
                        out_acc[:, s - 1 : s + 1, :],
                    )

            # h0 attention with h1's G/rotation/cheb interleaved (their
            # elementwise runs on Pool/Act; DVE carries the exp stream)
            av0 = [
                ph.tile([P, 6 if j < 2 else 4, 65], F32, tag=f"bank{j}",
                        name=f"av0{j}")
                for j in range(3)
            ]
            def h0_interleave(gkt):
                # history-key projections moved inside attention: their
                # engine-queue work lands behind the first exps. Each kc's
                # chunk2 khi-dup follows its projection immediately; the
                # rotated kt order first touches kt0 at unit 16 (gkt 3).
                if 0 <= gkt <= 3:
                    kc = gkt
                    emit_k(kc, pk=pr.tile([P, 512], F32, tag="sp", name="pk1"))
                    ks = slice(4 * kc, 4 * kc + 4)
                    nc.scalar.dma_start(
                        SgA[0][0:D, 2, ks, :], SgA[0][0:D, 0, ks, :]
                    )
                if gkt == 2:
                    emit_v(0, 0, pv=pr.tile([P, 512], F32, tag="sp", name="pv1"))
                elif gkt == 3:
                    emit_v(0, 1, pv=pr.tile([P, 512], F32, tag="sp", name="pv1"))
                elif gkt == 4:
                    emit_v(1, 0, pv=pr.tile([P, 512], F32, tag="sp", name="pv1"))
                elif gkt == 5:
                    emit_v(1, 1, pv=pr.tile([P, 512], F32, tag="sp", name="pv1"))
                elif gkt == 6:
                    # h1's psi tables + dups, now that h0's attention flows
                    emit_sga_tables(1)
                    nc.scalar.dma_start(M[1][D:P, 3, :, :], M[1][D:P, 2, :, :])
                    nc.scalar.dma_start(SgA[1][D:P, 2, :, :], SgA[1][D:P, 0, :, :])
                # h1 prep: one G chunk per site, rotation after its 4 chunks,
                # cheb (one pr slot) once the Pool finishing ops drained
                elif 10 <= gkt <= 25:
                    u, j = divmod(gkt - 10, 4)
                    emit_uw_g_chunk(
                        1, u, j,
                        h1buf[u][:, 0:2, :].rearrange("p a b -> p (a b)"),
                        h1buf[u][:, 2:4, :].rearrange("p a b -> p (a b)"),
                    )
                if gkt in (15, 19, 23, 27):
                    u = (15, 19, 23, 27).index(gkt)
                    emit_uw_rot(
                        1, u,
                        h1buf[u][:, 0:2, :].rearrange("p a b -> p (a b)"),
                        h1buf[u][:, 2:4, :].rearrange("p a b -> p (a b)"),
                        h1buf[u][:, 4:6, :],
                    )
                if gkt in (18, 22, 26, 30):
                    u = (18, 22, 26, 30).index(gkt)
                    emit_uw_cheb(
                        1, u, h1buf[u][:, 4:6, :],
                        pc=pr.tile([P, 512], F32, tag="sp", name="pc1"),
                    )
                # h0's per-qc z chunks as each query chunk's accumulation ends
                if gkt in (9, 17, 25):
                    emit_z_qc(0, av0, (gkt - 9) // 8)

            pend0 = []
            _u0 = [0]
            for qc in range(NQC):
                for kti in range(NKT):
                    kt = (kti + NKT // 2) % NKT
                    emit_unit(0, kt, kti, qc, av0, pend0)
                    _u0[0] += 1
                    if _u0[0] % 4 == 0:
                        h0_interleave(_u0[0] // 4 - 1)

            emit_av_flush(0, av0, pend0)
            emit_z_qc(0, av0, 3)
            cs1 = slice(0, NT)
            nc.scalar.dma_start(M[1][cs1, 4, :, :], M[1][cs1, 2, :, :])

            # h1 attention with h0's transpose + output projection streamed
            # (out tile s at gkt = 6 + 3s//2, i.e. 2 tiles per 3 sites)
            _out_sched = {6 + (3 * s) // 2: s for s in range(NS)}
            av1 = [
                ph.tile([P, 6 if j < 2 else 4, 65], F32, tag=f"bank{j}",
                        name=f"av1{j}")
                for j in range(3)
            ]

            def h1_interleave(gkt):
                if 1 <= gkt <= 8:
                    emit_z_tr(0, gkt - 1)
                if gkt in _out_sched:
                    emit_out_s(0, _out_sched[gkt])
                # h1's own per-qc tail chunks stream during later qcs
                if gkt in (10, 18, 26):
                    c = (gkt - 10) // 8
                    emit_z_qc(1, av1, c)
                elif gkt in (11, 19, 27):
                    c = (gkt - 11) // 8
                    emit_z_tr_pe(1, 2 * c, nc.vector.tensor_copy)
                elif gkt in (12, 20, 28):
                    c = (gkt - 12) // 8
                    emit_out_s(1, 4 * c)
                    emit_out_s(1, 4 * c + 1)
                elif gkt in (13, 21, 29):
                    c = (gkt - 13) // 8
                    emit_z_tr_pe(1, 2 * c + 1, nc.scalar.copy)
                elif gkt in (14, 22, 30):
                    c = (gkt - 14) // 8
                    emit_out_s(1, 4 * c + 2)
                    emit_out_s(1, 4 * c + 3)

            pend1 = []
            _u1 = [0]
            for qc in range(NQC):
                for kti in range(NKT):
                    kt = (kti + NKT // 2) % NKT
                    emit_unit(1, kt, kti, qc, av1, pend1)
                    _u1[0] += 1
                    if _u1[0] % 4 == 0:
                        h1_interleave(_u1[0] // 4 - 1)
            # tail: only the last query chunk's drain remains
            emit_av_flush(1, av1, pend1)
            emit_z_qc(1, av1, 3)
            emit_z_tr_pe(1, 6, nc.vector.tensor_copy)
            emit_out_s(1, 12)
            emit_out_s(1, 13)
            emit_z_tr_pe(1, 7, nc.scalar.copy)
            emit_out_s(1, 14)
            emit_out_s(1, 15)

    nc.compile()
    return nc


_NC_CACHE = None


def _get_program():
    global _NC_CACHE
    if _NC_CACHE is None:
        _NC_CACHE = build_program()
    return _NC_CACHE


def _fp8_hl(x):
    hi = np.clip(np.asarray(x, np.float32), -240, 240).astype(ml_dtypes.float8_e4m3)
    lo = np.clip(
        np.asarray(x, np.float32) - hi.astype(np.float32), -240, 240
    ).astype(ml_dtypes.float8_e4m3)
    return hi, lo


def make_in_maps(x, history, w_q, w_k, w_v, w_kr, w_o, u_bias, v_bias):
    bf = ml_dtypes.bfloat16
    all_x = np.concatenate([history, x], axis=1)  # [B, HpN, E]

    inv_freq = 1.0 / (10000.0 ** (np.arange(0, E, 2, dtype=np.float64) / E))  # [256]
    ang_f = np.outer(inv_freq[:128], np.arange(HpN, dtype=np.float64) - H)
    xn = (np.arange(HpN, dtype=np.float64) - H) / 2048.0
    T = np.polynomial.chebyshev.chebvander(xn, NT - 1)  # [HpN, NT]
    ang_s = np.outer(xn * 2048.0, inv_freq[128:256])  # [HpN, 128]
    tgt = np.concatenate([np.sin(ang_s), np.cos(ang_s)], axis=1)  # [HpN, 256]
    coef, *_ = np.linalg.lstsq(T, tgt, rcond=None)  # [NT, 256]
    sc = np.ascontiguousarray(coef.T)  # [256, NT]: rows 0-127 sin, 128-255 cos

    sin_hi, _ = _fp8_hl(np.sin(ang_f))
    cos_hi, _ = _fp8_hl(np.cos(ang_f))
    T_hi, T_lo = _fp8_hl(T.T)  # [NT, HpN]
    sin_f = sin_hi.astype(np.float32)
    cos_f = cos_hi.astype(np.float32)
    # SgF partition-major: [p][t][c][j], chunks c = [sin_hi, cos_hi]
    psiF = np.ascontiguousarray(
        np.stack(
            [sin_f.reshape(P, NKT, P), cos_f.reshape(P, NKT, P)], axis=2
        ).reshape(P, NKT * 2 * P)
    )
    # shared cheb T basis [p(64)][hi/lo][t][j] (device places it per head)
    psiT = np.ascontiguousarray(
        np.stack(
            [
                T_hi.astype(np.float32).reshape(NT, NKT, P),
                T_lo.astype(np.float32).reshape(NT, NKT, P),
            ],
            axis=1,
        ).reshape(NT, 2 * NKT * P)
    )
    # fast-psi half-compensation stationary [cos_hi(f0:64)|sin_hi(f64:128)]:
    # pairs with M chunk 5 = [Wlo(f0:64)|Ulo(f64:128)]
    psiC = np.ascontiguousarray(
        np.concatenate([cos_f[0:D], sin_f[D:P]], axis=0).reshape(P, NKT * P)
    )

    ang_b = np.outer(inv_freq, np.arange(N, dtype=np.float64))  # [256, N]
    rot = np.ascontiguousarray(
        np.concatenate([np.cos(ang_b), np.sin(ang_b)]).astype(bf)
    )  # [512, N]: rows 0:128 cos-fast, 128:256 cos-slow, 256:384 sin-fast, ...

    clip8 = lambda a: np.clip(a, -240, 240).astype(ml_dtypes.float8_e4m3)

    in_maps = []
    for c in range(N_CORES):
        b = c // 4
        h0 = HEADS_PER_CORE * (c % 4)
        axT = np.ascontiguousarray(all_x[b].T).astype(bf)
        wq2 = np.concatenate([w_q[h0], w_q[h0 + 1]], axis=1).astype(bf)  # [E, 128]
        wk2 = np.concatenate([w_k[h0], w_k[h0 + 1]], axis=1).astype(bf)
        wv2 = np.concatenate([w_v[h0], w_v[h0 + 1]], axis=1).astype(bf)
        wkrT = np.concatenate(
            [w_kr[h0].T, w_kr[h0 + 1].T], axis=0
        ).astype(bf)  # [128, E]: rows 0:64 = head0 (d), 64:128 = head1
        wo1h = np.stack([w_o[h0], w_o[h0 + 1]], axis=1).reshape(D, 2 * E)
        wo2 = np.concatenate([wo1h, wo1h], axis=0).astype(bf)  # [P, 2E]
        in_maps.append(
            {
                "axT": axT,
                "rot": rot,
                "psiF": clip8(psiF),
                "psiT": clip8(psiT),
                "psiC": clip8(psiC),
                "sc": np.ascontiguousarray(sc).astype(bf),
                "wq2": np.ascontiguousarray(wq2),
                "wk2": np.ascontiguousarray(wk2),
                "wv2": np.ascontiguousarray(wv2),
                "wkrT": np.ascontiguousarray(wkrT),
                "wo2": np.ascontiguousarray(wo2),
                "ub2": np.ascontiguousarray(
                    np.concatenate([u_bias[h0], u_bias[h0 + 1]]).reshape(P, 1)
                ).astype(np.float32),
                "vb2": np.ascontiguousarray(
                    np.concatenate([v_bias[h0], v_bias[h0 + 1]]).reshape(P, 1)
                ).astype(np.float32),
            }
        )
    return in_maps


def run(inputs, trace=False, **kw):
    from concourse.bass_utils import run_bass_kernel_spmd

    nc = _get_program()
    in_maps = make_in_maps(
        np.asarray(inputs["x"], np.float32),
        np.asarray(inputs["history"], np.float32),
        np.asarray(inputs["w_q"], np.float32),
        np.asarray(inputs["w_k"], np.float32),
        np.asarray(inputs["w_v"], np.float32),
        np.asarray(inputs["w_kr"], np.float32),
        np.asarray(inputs["w_o"], np.float32),
        np.asarray(inputs["u_bias"], np.float32),
        np.asarray(inputs["v_bias"], np.float32),
    )
    res = run_bass_kernel_spmd(nc, in_maps, list(range(N_CORES)), trace=trace, **kw)
    out = np.zeros((B, N, E), np.float32)
    for c in range(N_CORES):
        out[c // 4] += res.results[c]["oA"].astype(np.float32).reshape(N, E)
        out[c // 4] += res.results[c]["oB"].astype(np.float32).reshape(N, E)
    return out, res


def kernel(**inputs):
    # mask is all ones (per the problem spec), so score masking is a no-op
    # and the tensor is ignored.
    out, _ = run(inputs, trace=False)
    return out



# revision 133
# speedup vs baseline: 1.0104x; 1.0016x over previous
"""Transformer-XL multi-head self-attention on 8 Trainium2 NeuronCores.

Sharding: core c handles batch b = c//4 and heads {2*(c%4), 2*(c%4)+1}
(data-parallel over B x tensor-parallel over heads). Each core produces a
partial [N, E] output (its heads' w_o contributions); the host sums the 4
partials per batch element.

The XL relative-position term BD[i,j] = (q_i+v)·BDk[j-i+N-1] is computed
without the rel_shift gather via per-query rotation (angle-difference
identities): BD^T = Psi @ UW with Psi a shape-derived constant basis
(128 exact sin rows + 128 exact cos rows + 64 Chebyshev rows for the slow
frequencies) and UW per-query rotated coefficients.

Scores run on the PE in fp8e4 DoubleRow mode (0.5 cycles/row in the cost
model) with hi/lo error compensation: a bf16-accurate operand x is split
as x = hi + lo with hi = fp8(x), lo = fp8(x - hi), keeping selected cross
terms. Per 128-key tile the contraction is 6 chunks of 128 rows consumed
by 3 DoubleRow calls:
  [sin|cos]x[Uhi|Whi],
  [khi|Thi]x[qhi|chi], [klo|Thi]x[qhi|clo], [khi|Tlo]x[qlo|chi], pad
where T/c are the Chebyshev basis/coefficients and k/q carry the content
term (q+u)·k. The U/W (fast psi coefficient) lo-compensation is dropped
(one-sided both psi and U/W): host-side simulation puts the end-to-end
max-rel error at ~1.4% vs the 2% gate (vs ~1.2% with the compensation).
The value path (exp, V, attn@V, output projection) stays in bf16: fp8
noise there does not average out. exp is spread over Act/DVE/Pool
(Schraudolph on DVE/Pool; the extra Schraudolph noise is ~free: ~1.47%
even if every tile uses it).
"""

import sys

sys.path.insert(0, "/opt/trn_rl_repo")

import ml_dtypes
import numpy as np

import concourse.bass as bass
import concourse.mybir as mybir
from concourse import bacc
from concourse.masks import make_identity
from concourse.tile import TileContext

F32 = mybir.dt.float32
BF16 = mybir.dt.bfloat16
FP8 = mybir.dt.float8e4
I16 = mybir.dt.int16
AF = mybir.ActivationFunctionType
ALU = mybir.AluOpType
DR = mybir.MatmulPerfMode.DoubleRow

B, N, H, E, NH, D = 2, 2048, 2048, 512, 8, 64
HpN = H + N  # 4096
P = 128
NKT = HpN // P  # 32 key tiles
NPAIR = NKT // 2  # 16 key-tile pairs
NQC = N // 512  # 4 query chunks of 512
NEC = E // P  # 4 contraction chunks over E
NS = N // P  # 16 output row tiles
NT = 64  # chebyshev terms
HEADS_PER_CORE = 2
N_CORES = 8

LOG2E = 1.4426950408889634
SCORE_SHIFT = 1.5  # exp(s - c): cancels in softmax, bounds exp values
# exp tile engine rotation: (ctr % MOD) -> r < EXP_ACT on Act (exact),
# rest on DVE (Schraudolph). GPSIMD cannot read PSUM so Pool is out.
# Strict alternation: consecutive units' exps overlap across the two
# engines (each engine sees one ~1.1us exp per two 858ns PE units).
EXP_MOD, EXP_ACT = 2, 1


def build_program():
    nc = bacc.Bacc("TRN2", target_bir_lowering=False, debug=False)

    axT_d = nc.declare_dram_parameter("axT", [E, HpN], BF16, isOutput=False)
    rot_d = nc.declare_dram_parameter("rot", [E, N], BF16, isOutput=False)
    # SgF: shared fast-psi chunks, partition-major [p][t][c][j] so the DMA is
    # an identity layout with 8KB per-partition runs
    psiF_d = nc.declare_dram_parameter("psiF", [P, NKT * 2 * P], FP8, isOutput=False)
    # shared cheb T basis rows [p(64)][hi/lo][t][j]; identical for both heads
    # (placed at opposite partition halves on device)
    psiT_d = nc.declare_dram_parameter("psiT", [NT, 2 * NKT * P], FP8, isOutput=False)
    # fast-psi half-compensation stationary [cos_hi(f0:64)|sin_hi(f64:128)],
    # shared by both heads: [p][t][j]
    psiC_d = nc.declare_dram_parameter("psiC", [P, NKT * P], FP8, isOutput=False)
    sc_d = nc.declare_dram_parameter("sc", [2 * P, NT], BF16, isOutput=False)
    wq2_d = nc.declare_dram_parameter("wq2", [E, P], BF16, isOutput=False)
    wk2_d = nc.declare_dram_parameter("wk2", [E, P], BF16, isOutput=False)
    wv2_d = nc.declare_dram_parameter("wv2", [E, P], BF16, isOutput=False)
    wkrT_d = nc.declare_dram_parameter("wkrT", [P, E], BF16, isOutput=False)
    # wo duplicated on both partition halves (odd numT s-tiles live at 64:128)
    wo2_d = nc.declare_dram_parameter("wo2", [P, 2 * E], BF16, isOutput=False)
    ub2_d = nc.declare_dram_parameter("ub2", [P, 1], F32, isOutput=False)
    vb2_d = nc.declare_dram_parameter("vb2", [P, 1], F32, isOutput=False)
    # two per-head partial outputs (host sums): h0 streams during h1's
    # attention; h1 drains at the tail
    oA_d = nc.declare_dram_parameter("oA", [N, E], BF16, isOutput=True)
    oB_d = nc.declare_dram_parameter("oB", [N, E], BF16, isOutput=True)

    with TileContext(nc) as tc:
        with (
            tc.tile_pool(name="persist", bufs=1) as persist,
            tc.tile_pool(name="gst", bufs=4) as gst,       # G copies stream
            tc.tile_pool(name="mst", bufs=2) as mst,       # rotation temps
            tc.tile_pool(name="est", bufs=8) as est,       # exp tiles
            tc.tile_pool(name="dram", bufs=1, space="DRAM") as dram_pool,
            tc.tile_pool(name="pr", bufs=5, space="PSUM") as pr,   # 5x [P,512]
            tc.tile_pool(name="ph", bufs=1, space="PSUM") as ph,   # 3x [P,512]
        ):
            _sm = [0]

            def small_psum(shape, name, dtype=F32, tag=None):
                if tag is None:
                    i = _sm[0] % 3
                    _sm[0] += 1
                    tag = f"bank{i}"
                return ph.tile(shape, dtype, tag=tag, name=name)

            # ---------------- DMAs ----------------
            # One prioritized stream on the sync queue: the DMA engines are a
            # serialized resource, so emission order here IS the priority.
            # q proj needs {wq2, x-half}; the uw chain adds {wkr, rot, sc};
            # emit_k(4..7)/emit_v(x) add {wk2, wv2}; history keys come next,
            # then the attention-only psi tables and wo.
            wq2_s = persist.tile([P, NEC, P], BF16, tag="wq2")
            nc.sync.dma_start(wq2_s[:], wq2_d[:].rearrange("(c p) d -> p c d", p=P))
            # first x piece immediately after wq2 -- the small-weight DMAs'
            # per-transfer HWDGE overheads would delay the very first matmul
            axT_s = persist.tile([P, NEC, HpN], BF16, tag="axT", name="axT")
            axT = [axT_s[:, c, :] for c in range(NEC)]

            def x_piece(r):
                ks = slice(H + r * 512, H + (r + 1) * 512)
                nc.sync.dma_start(
                    axT_s[:, :, ks],
                    axT_d[:, ks].rearrange("(c p) k -> p c k", p=P),
                )

            x_piece(0)
            ub_s = persist.tile([P, 1], F32, tag="ub")
            nc.sync.dma_start(ub_s[:], ub2_d[:])
            vb_s = persist.tile([P, 1], F32, tag="vb")
            nc.sync.dma_start(vb_s[:], vb2_d[:])
            # wkr stacked on partitions: rows 0:64 = head0 d, 64:128 = head1 d
            wkr_s = persist.tile([P, NEC, P], BF16, tag="wkr")
            nc.sync.dma_start(
                wkr_s[:], wkrT_d[:].rearrange("p (c e) -> p c e", c=NEC)
            )
            for r in range(1, 4):
                x_piece(r)
            wk2_s = persist.tile([P, NEC, P], BF16, tag="wk2")
            nc.sync.dma_start(wk2_s[:], wk2_d[:].rearrange("(c p) d -> p c d", p=P))
            wv2_s = persist.tile([P, NEC, P], BF16, tag="wv2")
            nc.sync.dma_start(wv2_s[:], wv2_d[:].rearrange("(c p) d -> p c d", p=P))
            # slow rot rows first (they gate the cheb chain); fast rows feed
            # the deferred M-finish ops
            rot_s = persist.tile([P, 4, N], BF16, tag="rot")
            nc.sync.dma_start(rot_s[:, 1, :], rot_d[P : 2 * P, :])
            nc.sync.dma_start(rot_s[:, 3, :], rot_d[3 * P : 4 * P, :])
            nc.sync.dma_start(rot_s[:, 0, :], rot_d[0:P, :])
            nc.sync.dma_start(rot_s[:, 2, :], rot_d[2 * P : 3 * P, :])
            sc_s = persist.tile([P, 2, NT], BF16, tag="sc")
            nc.sync.dma_start(sc_s[:], sc_d[:].rearrange("(k p) r -> p k r", p=P))
            SgF = persist.tile([P, NKT, 2, P], FP8, tag="SgF")
            nc.sync.dma_start(
                SgF[:], psiF_d[:].rearrange("p (t c j) -> p t c j", c=2, j=P)
            )
            # SgA free layout is chunk-major [c][t][j] so partition-sliced
            # chunk DMAs have 4KB contiguous runs. Only h0's tables load in
            # phase A; h1's are deferred past h0's attention start (the DMA
            # engines are a serialized resource on the startup critical path).
            SgA = []
            for h in range(HEADS_PER_CORE):
                t = persist.tile([P, 4, NKT, P], FP8, tag=f"SgA{h}", name=f"SgA{h}")
                SgA.append(t)

            def emit_sga_tables(h):
                tp = (1 - h) * D
                tps = slice(tp, tp + NT)
                nc.sync.dma_start(
                    SgA[h][tps, 0, :, :],
                    psiT_d[:, 0 : NKT * P].rearrange("p (t j) -> p t j", j=P),
                )
                nc.sync.dma_start(
                    SgA[h][tps, 2, :, :],
                    psiT_d[:, NKT * P :].rearrange("p (t j) -> p t j", j=P),
                )
                nc.sync.dma_start(
                    SgA[h][:, 3, :, :],
                    psiC_d[:].rearrange("p (t j) -> p t j", j=P),
                )
                # chunk1's T-half duplicates chunk0's (device-side dup)
                nc.scalar.dma_start(SgA[h][tps, 1, :, :], SgA[h][tps, 0, :, :])

            emit_sga_tables(0)
            wo_s = persist.tile([P, 2, E], BF16, tag="wo")
            nc.sync.dma_start(wo_s[:], wo2_d[:].rearrange("p (h e) -> p h e", h=2))
            # history lands last: its keys are first needed ~4 units into
            # attention, well after the rot/psi-gated startup chain
            for r in range(4):
                ks = slice(r * 512, (r + 1) * 512)
                nc.sync.dma_start(
                    axT_s[:, :, ks],
                    axT_d[:, ks].rearrange("(c p) k -> p c k", p=P),
                )

            identb = persist.tile([P, P], BF16, tag="identb")
            make_identity(nc, identb[:])

            # ---------------- persistent compute tiles ----------------
            # M chunks per head: 0=Uhi 1=Whi 2=[qhi|chi]
            # 3=[qhi-dup|clo] 4=[qlo|chi-dup] 5=[Wlo(f 0:64)|Ulo(f 64:128)]
            # (chunk 5 pairs with the psiA half-compensation stationary
            # [cos_hi(0:64)|sin_hi(64:128)] in the otherwise-wasted pad slot)
            M = []
            for h in range(HEADS_PER_CORE):
                m = persist.tile([P, 6, NQC, 512], FP8, tag=f"M{h}", name=f"M{h}")
                M.append(m)
            qv_s = persist.tile([P, N], BF16, tag="qv_s")
            vo = []
            for h in range(HEADS_PER_CORE):
                v = persist.tile([P, NKT, 66], BF16, tag=f"vo{h}", name=f"vo{h}")
                nc.gpsimd.memset(v[:, :, 64:66], 0.0)
                nc.gpsimd.memset(v[:, :, 64:65], 1.0)
                vo.append(v)
            # numTT: query-major pre-scaled numerators [q, s, d] (z separate);
            # numT: d-major via 128x128 transposes of s-tile PAIRS -- even
            # s-tile's d on partitions 0:64, odd on 64:128
            numT = []
            numTT = []
            zcs = []
            for h in range(HEADS_PER_CORE):
                t = persist.tile(
                    [P, NS // 2, P], BF16, tag=f"numT{h}", name=f"numT{h}"
                )
                numT.append(t)
                tt = persist.tile(
                    [P, NS, D], BF16, tag=f"numTT{h}", name=f"numTT{h}"
                )
                numTT.append(tt)
                zcs.append(
                    persist.tile([P, NS], F32, tag=f"zc{h}", name=f"zc{h}")
                )
            out_acc = persist.tile([P, NS, E], BF16, tag="out_acc")
            nbias = persist.tile([P, 1], F32, tag="nbias")
            nc.vector.memset(nbias[:], -SCORE_SHIFT)

            # ---------------- phase A: projections ----------------
            # q projection, both heads packed, emitted chunk-outer so the PE
            # starts as soon as each axT chunk lands. pq psums use the ph
            # banks (free until the av accumulators take them).
            pqs = [small_psum([P, 512], f"pq{qc}") for qc in range(NQC)]
            for qc in range(NQC):
                for c in range(NEC):
                    nc.tensor.matmul(
                        pqs[qc][:],
                        wq2_s[:, c, :],
                        axT[c][:, H + qc * 512 : H + (qc + 1) * 512],
                        start=(c == 0),
                        stop=(c == NEC - 1),
                    )
            for qc in range(NQC):
                pq = pqs[qc]
                qs = slice(qc * 512, (qc + 1) * 512)
                nc.vector.tensor_scalar_add(qv_s[:, qs], pq[:], vb_s[:])
                for h in range(HEADS_PER_CORE):
                    hp = slice(h * D, (h + 1) * D)
                    nc.vector.tensor_scalar_add(
                        M[h][hp, 2, qc, :], pq[hp, :], ub_s[hp]
                    )
                    nc.vector.scalar_tensor_tensor(
                        M[h][hp, 4, qc, :], pq[hp, :], ub_s[hp],
                        M[h][hp, 2, qc, :], ALU.add, ALU.subtract,
                    )

            def emit_uw_g_chunk(h, qc, j, sfd, ssd):
                # G: e 0:128 sin-fast + 256:384 cos-fast (sf);
                #    e 128:256 sin-slow + 384:512 cos-slow (ss)
                # one 1-bank psum + copy per chunk so at most one score-stream
                # slot is borrowed at a time
                hp = slice(h * D, (h + 1) * D)
                qs = slice(qc * 512, (qc + 1) * 512)
                half, jj = j // 2, j % 2
                dst = sfd if half == 0 else ssd
                g = pr.tile([P, 512], F32, tag="sp", name="g")
                nc.tensor.matmul(
                    g[:], wkr_s[hp, 2 * jj + half, :], qv_s[hp, qs],
                    start=True, stop=True,
                )
                nc.scalar.copy(dst[:, jj * 512 : (jj + 1) * 512], g[:])

            def emit_uw_g(h, qc, sfd, ssd):
                for j in range(4):
                    emit_uw_g_chunk(h, qc, j, sfd, ssd)

            def emit_uw_rot_slow(h, qc, ss, usw, me, add_eng=None):
                # slow half: rotate; compression happens in emit_uw_cheb
                add_eng = add_eng or nc.gpsimd
                qs = slice(qc * 512, (qc + 1) * 512)
                m5 = mst.tile([P, 512], BF16, tag="m1", name="m5")
                m6 = mst.tile([P, 512], BF16, tag="m2", name="m6")
                m7 = mst.tile([P, 512], BF16, tag="m3", name="m7")
                m8 = mst.tile([P, 512], BF16, tag="m4", name="m8")
                me[4].tensor_mul(m5[:], ss[:, 0:512], rot_s[:, 1, qs])
                me[5].tensor_mul(m6[:], ss[:, 512:1024], rot_s[:, 3, qs])
                me[6].tensor_mul(m7[:], ss[:, 512:1024], rot_s[:, 1, qs])
                me[7].tensor_mul(m8[:], ss[:, 0:512], rot_s[:, 3, qs])
                add_eng.tensor_add(usw[:, 0, :], m5[:], m6[:])
                add_eng.tensor_sub(usw[:, 1, :], m7[:], m8[:])

            def emit_uw_rot_fast(h, qc, sf, ubf, wbf, me, add_eng=None):
                # fast half: U = G*cos + Gc*sin ; W = Gc*cos - G*sin
                add_eng = add_eng or nc.gpsimd
                qs = slice(qc * 512, (qc + 1) * 512)
                m1 = mst.tile([P, 512], BF16, tag="m1")
                m2 = mst.tile([P, 512], BF16, tag="m2")
                m3 = mst.tile([P, 512], BF16, tag="m3")
                m4 = mst.tile([P, 512], BF16, tag="m4")
                me[0].tensor_mul(m1[:], sf[:, 0:512], rot_s[:, 0, qs])
                me[1].tensor_mul(m2[:], sf[:, 512:1024], rot_s[:, 2, qs])
                me[2].tensor_mul(m3[:], sf[:, 512:1024], rot_s[:, 0, qs])
                me[3].tensor_mul(m4[:], sf[:, 0:512], rot_s[:, 2, qs])
                add_eng.tensor_add(ubf[:], m1[:], m2[:])
                add_eng.tensor_sub(wbf[:], m3[:], m4[:])

            def emit_uw_rot_finish(h, qc, ubf, wbf, c0, c1, s5):
                c0(M[h][:, 0, qc, :], ubf[:])
                c1(M[h][:, 1, qc, :], wbf[:])
                # half lo-comp into the pad slot (partition-aligned halves)
                s5(M[h][0:D, 5, qc, :], wbf[0:D, :], M[h][0:D, 1, qc, :])
                s5(M[h][D:P, 5, qc, :], ubf[D:P, :], M[h][D:P, 0, qc, :])

            def emit_uw_rot(h, qc, sf, ss, usw):
                # combined form used for h1 during h0's attention: muls all
                # DVE (fast, 3/8 exps there), everything downstream of a mul
                # on Pool so the DVE queue never waits cross-engine.
                V, G = nc.vector, nc.gpsimd
                me = (V,) * 8
                ubf = mst.tile([P, 512], BF16, tag="ubf")
                wbf = mst.tile([P, 512], BF16, tag="wbf")
                emit_uw_rot_slow(h, qc, ss, usw, me)
                emit_uw_rot_fast(h, qc, sf, ubf, wbf, me)
                emit_uw_rot_finish(
                    h, qc, ubf, wbf, G.tensor_copy, G.tensor_copy, G.tensor_sub
                )

            def emit_uw_cheb(h, qc, usw, pc=None):
                # cheb coefs land on the head's opposite partition half
                po = (1 - h) * D
                cs = slice(po, po + NT)
                if pc is None:
                    pc = small_psum([P, 512], "pc")
                for k in range(2):
                    nc.tensor.matmul(
                        pc[cs, :], sc_s[:, k, :], usw[:, k, :],
                        start=(k == 0), stop=(k == 1),
                    )
                nc.scalar.copy(M[h][cs, 2, qc, :], pc[cs, :])
                nc.vector.tensor_sub(
                    M[h][cs, 3, qc, :], pc[cs, :], M[h][cs, 2, qc, :]
                )

            def emit_k(kc, pk=None):
                if pk is None:
                    pk = small_psum([P, 512], "pk")
                for c in range(NEC):
                    nc.tensor.matmul(
                        pk[:],
                        wk2_s[:, c, :],
                        axT[c][:, kc * 512 : (kc + 1) * 512],
                        start=(c == 0),
                        stop=(c == NEC - 1),
                    )
                ks = slice(4 * kc, 4 * kc + 4)
                for h in range(HEADS_PER_CORE):
                    hp = slice(h * D, (h + 1) * D)
                    pkv = pk[hp, :].rearrange("p (t j) -> p t j", j=P)
                    nc.scalar.copy(SgA[h][hp, 0, ks, :], pkv)
                    nc.vector.tensor_sub(
                        SgA[h][hp, 1, ks, :], pkv, SgA[h][hp, 0, ks, :]
                    )

            def emit_v(h, g, pv=None):
                hs = slice(h * D, (h + 1) * D)
                if pv is None:
                    pv = small_psum([P, 512], "pv")
                for k8 in range(8):
                    kt = g * 8 + k8
                    for c in range(NEC):
                        nc.tensor.matmul(
                            pv[:, k8 * D : (k8 + 1) * D],
                            axT[c][:, kt * P : (kt + 1) * P],
                            wv2_s[:, c, hs],
                            start=(c == 0),
                            stop=(c == NEC - 1),
                        )
                cp = nc.scalar.copy if (h + g) % 2 else nc.vector.tensor_copy
                cp(
                    vo[h][:, g * 8 : (g + 1) * 8, 0:D],
                    pv[:].rearrange("p (t d) -> p t d", d=D),
                )

            # h0 UW fully in phase A (streaming); h1's G copies land in a
            # persistent tile recycled from axT's tag so h1's rotation
            # (engine-only) can run during h0's attention.
            h1b = persist.tile(
                [P, NQC, 6, 512], BF16, tag="axT", name="h1buf"
            )
            h1buf = [h1b[:, u, :, :] for u in range(NQC)]

            # Phase A PE order: all G matmuls (only need q), then x-key
            # projections (their axT DMA lands early), then history keys,
            # then chebs (gated on the slow-rot chain) and finally the M
            # finish ops -- emitted last so the DVE queue never blocks the
            # attention exps behind a Pool dependency.
            # Phase A emission: q -> x-keys -> all G's -> x-values -> slow
            # rotations (DVE, matching the slow-first rot DMA) -> fast
            # rotations -> chebs -> finish. All same-queue chains; the only
            # cross-engine hops (ubf/wbf on Pool, M5 on Pool behind them)
            # are off the DVE queue so the attention exps aren't blocked.
            V, G, A = nc.vector, nc.gpsimd, nc.scalar
            me0 = (V,) * 8
            uwt = []
            for u in range(NQC):
                sf = gst.tile([P, 1024], BF16, tag="sf")
                ss = gst.tile([P, 1024], BF16, tag="ss")
                usw = gst.tile([P, 2, 512], BF16, tag="usw")
                ubf = gst.tile([P, 512], BF16, tag="ubf")
                wbf = gst.tile([P, 512], BF16, tag="wbf")
                uwt.append((sf, ss, usw, ubf, wbf))
            # x-key projections only in phase A; history keys are emitted
            # inside early h0 attention (their engine-queue work then sits
            # behind the first exps instead of gating them)
            for u in range(NQC):
                emit_k(4 + u)
            nc.scalar.dma_start(SgA[0][0:D, 2, 16:32, :], SgA[0][0:D, 0, 16:32, :])
            for u in range(NQC):
                emit_uw_g(0, u, uwt[u][0][:], uwt[u][1][:])
            for g in (2, 3):
                emit_v(0, g)
                emit_v(1, g)
            for u in range(NQC):
                emit_uw_rot_slow(0, u, uwt[u][1], uwt[u][2], me0, add_eng=V)
            for u in range(NQC):
                emit_uw_rot_fast(0, u, uwt[u][0], uwt[u][3], uwt[u][4], me0)
            # dups via DMA (off-engine): M chunk 3 q-half <- chunk 2 q-half;
            # chunk 4 cheb-half dups are per-qc so attention(qc0) only gates
            # on u=0's chain.
            nc.scalar.dma_start(M[0][0:D, 3, :, :], M[0][0:D, 2, :, :])
            cs0 = slice(D, D + NT)
            for u in range(NQC):
                emit_uw_cheb(0, u, uwt[u][2])
                nc.scalar.dma_start(
                    M[0][cs0, 4, u, :], M[0][cs0, 2, u, :]
                )
            for u in range(NQC):
                emit_uw_rot_finish(
                    0, u, uwt[u][3], uwt[u][4],
                    A.copy, A.copy, V.tensor_sub,
                )

            # ---------------- phase B: attention ----------------
            # Unit = one (key tile, query chunk): score psum is a 1-bank
            # [P, 512] tile from the 5-deep pr pool, so the
            # ps -> exp -> frees-slot chain never stalls the PE. exp
            # alternates Act (exact) / DVE (Schraudolph) per unit; during
            # h0's attention DVE also carries h1's rotation, so it only
            # takes 3 of 8 exps there.
            _expctr = [0]
            _dve_exp = {0: (1, 3, 5, 7), 1: (1, 3, 5, 7)}

            def emit_av(h, kt, kti, qc, pE, avv):
                for qt in range(4):
                    qg = qc * 4 + qt
                    bk, sl = divmod(qg, 6)
                    nc.tensor.matmul(
                        avv[bk][:, sl, :],
                        pE[:, qt * P : (qt + 1) * P],
                        vo[h][:, kt, 0:65],
                        start=(kti == 0 and qg in (0, 6, 12)),
                        stop=(kti == NKT - 1 and qg in (5, 11, 15)),
                        skip_group_check=True,
                    )

            def emit_unit(h, kt, kti, qc, avv, pend):
                ps = pr.tile([P, 512], F32, tag="sp", name="ps")
                nc.tensor.matmul(
                    ps[:], SgF[:, kt, :, :], M[h][:, 0:2, qc, :],
                    start=True, stop=False, perf_mode=DR,
                )
                nc.tensor.matmul(
                    ps[:], SgA[h][:, 0:2, kt, :], M[h][:, 2:4, qc, :],
                    start=False, stop=False, perf_mode=DR,
                )
                nc.tensor.matmul(
                    ps[:], SgA[h][:, 2:4, kt, :], M[h][:, 4:6, qc, :],
                    start=False, stop=True, perf_mode=DR,
                )
                if len(pend) >= 4:
                    emit_av(h, *pend.pop(0), avv)
                et = est.tile([P, 512], BF16, tag="E")
                if _expctr[0] % 8 not in _dve_exp[h]:
                    nc.scalar.activation(
                        et[:], ps[:], AF.Exp, scale=0.125, bias=nbias[:]
                    )
                else:
                    # Schraudolph: int16 bits = 128*(log2e*(s/8 - c) + 127)
                    nc.vector.tensor_scalar(
                        et[:].bitcast(I16), ps[:],
                        0.125 * P * LOG2E,
                        P * 127.0 - SCORE_SHIFT * P * LOG2E - 8.5,
                        ALU.mult, ALU.add,
                    )
                _expctr[0] += 1
                pend.append((kt, kti, qc, et))

            def emit_av_flush(h, avv, pend):
                while pend:
                    emit_av(h, *pend.pop(0), avv)

            zrecs = [
                persist.tile([P, NS], F32, tag=f"zrec{h}", name=f"zrec{h}")
                for h in range(HEADS_PER_CORE)
            ]

            def emit_z_scale(h, avv, s):
                # write numTT PRE-SCALED by 1/z (per-partition scalar per
                # s-tile) so the out-projection result needs no scaling
                bk, sl = divmod(s, 6)
                if s % 2 == 0:
                    nc.scalar.activation(
                        numTT[h][:, s, :], avv[bk][:, sl, 0:D], AF.Copy,
                        scale=zrecs[h][:, s : s + 1],
                    )
                else:
                    nc.vector.tensor_scalar_mul(
                        numTT[h][:, s, :], avv[bk][:, sl, 0:D],
                        zrecs[h][:, s : s + 1],
                    )

            def emit_z_qc(h, avv, qc):
                # one query chunk's denominators + pre-scaled numerators,
                # streamable as soon as that chunk's accumulation stops
                zc = zcs[h]
                s4 = slice(4 * qc, 4 * qc + 4)
                b0, l0 = divmod(4 * qc, 6)
                if l0 + 4 <= 6:
                    nc.vector.tensor_copy(zc[:, s4], avv[b0][:, l0 : l0 + 4, 64])
                else:
                    k = 6 - l0
                    nc.vector.tensor_copy(
                        zc[:, 4 * qc : 4 * qc + k], avv[b0][:, l0:6, 64]
                    )
                    nc.vector.tensor_copy(
                        zc[:, 4 * qc + k : 4 * qc + 4],
                        avv[b0 + 1][:, 0 : 4 - k, 64],
                    )
                nc.vector.reciprocal(zrecs[h][:, s4], zc[:, s4])
                for s in range(4 * qc, 4 * qc + 4):
                    emit_z_scale(h, avv, s)

            def emit_z_tr(h, s2):
                # transpose one PAIR of numerator s-tiles ([128,128] block)
                # back to d-major via the DMA xbar (off-engine)
                nc.sync.dma_start_transpose(
                    numT[h][:, s2, :],
                    numTT[h][:, 2 * s2 : 2 * s2 + 2, :],
                )

            def emit_z_tr_pe(h, s2, copy_eng):
                pz = pr.tile([P, P], BF16, tag="sp", name="pz")
                nc.tensor.transpose(
                    pz[:], numTT[h][:, 2 * s2 : 2 * s2 + 2, :], identb[:]
                )
                copy_eng(numT[h][:, s2, :], pz[:])

            def emit_out_s(h, s):
                # numT is pre-scaled by 1/z, so the psum->sbuf conversion is
                # a plain copy (alternating Act/DVE to spread the load)
                po = pr.tile([P, 512], F32, tag="sp", name="po")
                hp = (s % 2) * D
                nc.tensor.matmul(
                    po[:], numT[h][hp : hp + D, s // 2, :],
                    wo_s[hp : hp + D, h, :],
                    start=True, stop=True,
                )
                if s % 2 == 0:
                    nc.scalar.copy(out_acc[:, s, :], po[:])
                else:
                    nc.vector.tensor_copy(out_acc[:, s, :], po[:])
                if h == 0:
                    nc.sync.dma_start(
                        oA_d[:].rearrange("(s p) e -> p s e", p=P)[:, s, :],
                        out_acc[:, s, :],
                    )
                elif s in (3, 7, 11):
                    # h1 streams in 4-tile batches...
                    nc.sync.dma_start(
                        oB_d[:].rearrange("(s p) e -> p s e", p=P)[:, s - 3 : s + 1, :],
                        out_acc[:, s - 3 : s + 1, :],
                    )
                elif s in (13, 15):
                    # ...except the final quad goes as two pairs so the last
                    # transfer (the kernel's true tail) is half as long
                    nc.sync.dma_start(
                        oB_d[:].rearrange("(s p) e -> p s e", p=P)[:, s - 1 : s + 1, :],
                        out_acc[:, s - 1 : s + 1, :],
                    )

            # h0 attention with h1's G/rotation/cheb interleaved (their
            # elementwise runs on Pool/Act; DVE carries the exp stream)
            av0 = [
                ph.tile([P, 6 if j < 2 else 4, 65], F32, tag=f"bank{j}",
                        name=f"av0{j}")
                for j in range(3)
            ]
            def h0_interleave(gkt):
                # history-key projections moved inside attention: their
                # engine-queue work lands behind the first exps. Each kc's
                # chunk2 khi-dup follows its projection immediately; the
                # rotated kt order first touches kt0 at unit 16 (gkt 3).
                if 0 <= gkt <= 3:
                    kc = gkt
                    emit_k(kc, pk=pr.tile([P, 512], F32, tag="sp", name="pk1"))
                    ks = slice(4 * kc, 4 * kc + 4)
                    nc.scalar.dma_start(
                        SgA[0][0:D, 2, ks, :], SgA[0][0:D, 0, ks, :]
                    )
                if gkt == 2:
                    emit_v(0, 0, pv=pr.tile([P, 512], F32, tag="sp", name="pv1"))
                elif gkt == 3:
                    emit_v(0, 1, pv=pr.tile([P, 512], F32, tag="sp", name="pv1"))
                elif gkt == 4:
                    emit_v(1, 0, pv=pr.tile([P, 512], F32, tag="sp", name="pv1"))
                elif gkt == 5:
                    emit_v(1, 1, pv=pr.tile([P, 512], F32, tag="sp", name="pv1"))
                elif gkt == 6:
                    # h1's psi tables + dups, now that h0's attention flows
                    emit_sga_tables(1)
                    nc.scalar.dma_start(M[1][D:P, 3, :, :], M[1][D:P, 2, :, :])
                    nc.scalar.dma_start(SgA[1][D:P, 2, :, :], SgA[1][D:P, 0, :, :])
                # h1 prep: one G chunk per site, rotation after its 4 chunks,
                # cheb (one pr slot) once the Pool finishing ops drained
                elif 10 <= gkt <= 25:
                    u, j = divmod(gkt - 10, 4)
                    emit_uw_g_chunk(
                        1, u, j,
                        h1buf[u][:, 0:2, :].rearrange("p a b -> p (a b)"),
                        h1buf[u][:, 2:4, :].rearrange("p a b -> p (a b)"),
                    )
                if gkt in (15, 19, 23, 27):
                    u = (15, 19, 23, 27).index(gkt)
                    emit_uw_rot(
                        1, u,
                        h1buf[u][:, 0:2, :].rearrange("p a b -> p (a b)"),
                        h1buf[u][:, 2:4, :].rearrange("p a b -> p (a b)"),
                        h1buf[u][:, 4:6, :],
                    )
                if gkt in (18, 22, 26, 30):
                    u = (18, 22, 26, 30).index(gkt)
                    emit_uw_cheb(
                        1, u, h1buf[u][:, 4:6, :],
                        pc=pr.tile([P, 512], F32, tag="sp", name="pc1"),
                    )
                # h0's per-qc z chunks as each query chunk's accumulation ends
                if gkt in (9, 17, 25):
                    emit_z_qc(0, av0, (gkt - 9) // 8)

            pend0 = []
            _u0 = [0]
            for qc in range(NQC):
                for kti in range(NKT):
                    kt = (kti + NKT // 2) % NKT
                    emit_unit(0, kt, kti, qc, av0, pend0)
                    _u0[0] += 1
                    if _u0[0] % 4 == 0:
                        h0_interleave(_u0[0] // 4 - 1)

            emit_av_flush(0, av0, pend0)
            emit_z_qc(0, av0, 3)
            cs1 = slice(0, NT)
            nc.scalar.dma_start(M[1][cs1, 4, :, :], M[1][cs1, 2, :, :])

            # h1 attention with h0's transpose + output projection streamed
            # (out tile s at gkt = 6 + 3s//2, i.e. 2 tiles per 3 sites)
            _out_sched = {6 + (3 * s) // 2: s for s in range(NS)}
            av1 = [
                ph.tile([P, 6 if j < 2 else 4, 65], F32, tag=f"bank{j}",
                        name=f"av1{j}")
                for j in range(3)
            ]

            def h1_interleave(gkt):
                if 1 <= gkt <= 8:
                    emit_z_tr(0, gkt - 1)
                if gkt in _out_sched:
                    emit_out_s(0, _out_sched[gkt])
                # h1's own per-qc tail chunks stream during later qcs
                if gkt in (10, 18, 26):
                    c = (gkt - 10) // 8
                    emit_z_qc(1, av1, c)
                elif gkt in (11, 19, 27):
                    c = (gkt - 11) // 8
                    emit_z_tr_pe(1, 2 * c, nc.vector.tensor_copy)
                elif gkt in (12, 20, 28):
                    c = (gkt - 12) // 8
                    emit_out_s(1, 4 * c)
                    emit_out_s(1, 4 * c + 1)
                elif gkt in (13, 21, 29):
                    c = (gkt - 13) // 8
                    emit_z_tr_pe(1, 2 * c + 1, nc.scalar.copy)
                elif gkt in (14, 22, 30):
                    c = (gkt - 14) // 8
                    emit_out_s(1, 4 * c + 2)
                    emit_out_s(1, 4 * c + 3)

            pend1 = []
            _u1 = [0]
            for qc in range(NQC):
                for kti in range(NKT):
                    kt = (kti + NKT // 2) % NKT
                    emit_unit(1, kt, kti, qc, av1, pend1)
                    _u1[0] += 1
                    if _u1[0] % 4 == 0:
                        h1_interleave(_u1[0] // 4 - 1)
            # tail: only the last query chunk's drain remains
            emit_av_flush(1, av1, pend1)
            emit_z_qc(1, av1, 3)
            emit_z_tr_pe(1, 6, nc.vector.tensor_copy)
            emit_out_s(1, 12)
            emit_out_s(1, 13)
            emit_z_tr_pe(1, 7, nc.scalar.copy)
            emit_out_s(1, 14)
            emit_out_s(1, 15)

    nc.compile()
    return nc


_NC_CACHE = None


def _get_program():
    global _NC_CACHE
    if _NC_CACHE is None:
        _NC_CACHE = build_program()
    return _NC_CACHE


def _fp8_hl(x):
    hi = np.clip(np.asarray(x, np.float32), -240, 240).astype(ml_dtypes.float8_e4m3)
    lo = np.clip(
        np.asarray(x, np.float32) - hi.astype(np.float32), -240, 240
    ).astype(ml_dtypes.float8_e4m3)
    return hi, lo


def make_in_maps(x, history, w_q, w_k, w_v, w_kr, w_o, u_bias, v_bias):
    bf = ml_dtypes.bfloat16
    all_x = np.concatenate([history, x], axis=1)  # [B, HpN, E]

    inv_freq = 1.0 / (10000.0 ** (np.arange(0, E, 2, dtype=np.float64) / E))  # [256]
    ang_f = np.outer(inv_freq[:128], np.arange(HpN, dtype=np.float64) - H)
    xn = (np.arange(HpN, dtype=np.float64) - H) / 2048.0
    T = np.polynomial.chebyshev.chebvander(xn, NT - 1)  # [HpN, NT]
    ang_s = np.outer(xn * 2048.0, inv_freq[128:256])  # [HpN, 128]
    tgt = np.concatenate([np.sin(ang_s), np.cos(ang_s)], axis=1)  # [HpN, 256]
    coef, *_ = np.linalg.lstsq(T, tgt, rcond=None)  # [NT, 256]
    sc = np.ascontiguousarray(coef.T)  # [256, NT]: rows 0-127 sin, 128-255 cos

    sin_hi, _ = _fp8_hl(np.sin(ang_f))
    cos_hi, _ = _fp8_hl(np.cos(ang_f))
    T_hi, T_lo = _fp8_hl(T.T)  # [NT, HpN]
    sin_f = sin_hi.astype(np.float32)
    cos_f = cos_hi.astype(np.float32)
    # SgF partition-major: [p][t][c][j], chunks c = [sin_hi, cos_hi]
    psiF = np.ascontiguousarray(
        np.stack(
            [sin_f.reshape(P, NKT, P), cos_f.reshape(P, NKT, P)], axis=2
        ).reshape(P, NKT * 2 * P)
    )
    # shared cheb T basis [p(64)][hi/lo][t][j] (device places it per head)
    psiT = np.ascontiguousarray(
        np.stack(
            [
                T_hi.astype(np.float32).reshape(NT, NKT, P),
                T_lo.astype(np.float32).reshape(NT, NKT, P),
            ],
            axis=1,
        ).reshape(NT, 2 * NKT * P)
    )
    # fast-psi half-compensation stationary [cos_hi(f0:64)|sin_hi(f64:128)]:
    # pairs with M chunk 5 = [Wlo(f0:64)|Ulo(f64:128)]
    psiC = np.ascontiguousarray(
        np.concatenate([cos_f[0:D], sin_f[D:P]], axis=0).reshape(P, NKT * P)
    )

    ang_b = np.outer(inv_freq, np.arange(N, dtype=np.float64))  # [256, N]
    rot = np.ascontiguousarray(
        np.concatenate([np.cos(ang_b), np.sin(ang_b)]).astype(bf)
    )  # [512, N]: rows 0:128 cos-fast, 128:256 cos-slow, 256:384 sin-fast, ...

    clip8 = lambda a: np.clip(a, -240, 240).astype(ml_dtypes.float8_e4m3)

    in_maps = []
    for c in range(N_CORES):
        b = c // 4
        h0 = HEADS_PER_CORE * (c % 4)
        axT = np.ascontiguousarray(all_x[b].T).astype(bf)
        wq2 = np.concatenate([w_q[h0], w_q[h0 + 1]], axis=1).astype(bf)  # [E, 128]
        wk2 = np.concatenate([w_k[h0], w_k[h0 + 1]], axis=1).astype(bf)
        wv2 = np.concatenate([w_v[h0], w_v[h0 + 1]], axis=1).astype(bf)
        wkrT = np.concatenate(
            [w_kr[h0].T, w_kr[h0 + 1].T], axis=0
        ).astype(bf)  # [128, E]: rows 0:64 = head0 (d), 64:128 = head1
        wo1h = np.stack([w_o[h0], w_o[h0 + 1]], axis=1).reshape(D, 2 * E)
        wo2 = np.concatenate([wo1h, wo1h], axis=0).astype(bf)  # [P, 2E]
        in_maps.append(
            {
                "axT": axT,
                "rot": rot,
                "psiF": clip8(psiF),
                "psiT": clip8(psiT),
                "psiC": clip8(psiC),
                "sc": np.ascontiguousarray(sc).astype(bf),
                "wq2": np.ascontiguousarray(wq2),
                "wk2": np.ascontiguousarray(wk2),
                "wv2": np.ascontiguousarray(wv2),
                "wkrT": np.ascontiguousarray(wkrT),
                "wo2": np.ascontiguousarray(wo2),
                "ub2": np.ascontiguousarray(
                    np.concatenate([u_bias[h0], u_bias[h0 + 1]]).reshape(P, 1)
                ).astype(np.float32),
                "vb2": np.ascontiguousarray(
                    np.concatenate([v_bias[h0], v_bias[h0 + 1]]).reshape(P, 1)
                ).astype(np.float32),
            }
        )
    return in_maps


def run(inputs, trace=False, **kw):
    from concourse.bass_utils import run_bass_kernel_spmd

    nc = _get_program()
    in_maps = make_in_maps(
        np.asarray(inputs["x"], np.float32),
        np.asarray(inputs["history"], np.float32),
        np.asarray(inputs["w_q"], np.float32),
        np.asarray(inputs["w_k"], np.float32),
        np.asarray(inputs["w_v"], np.float32),
        np.asarray(inputs["w_kr"], np.float32),
        np.asarray(inputs["w_o"], np.float32),
        np.asarray(inputs["u_bias"], np.float32),
        np.asarray(inputs["v_bias"], np.float32),
    )
    res = run_bass_kernel_spmd(nc, in_maps, list(range(N_CORES)), trace=trace, **kw)
    out = np.zeros((B, N, E), np.float32)
    for c in range(N_CORES):
        out[c // 4] += res.results[c]["oA"].astype(np.float32).reshape(N, E)
        out[c // 4] += res.results[c]["oB"].astype(np.float32).reshape(N, E)
    return out, res


def kernel(**inputs):
    # mask is all ones (per the problem spec), so score masking is a no-op
    # and the tensor is ignored.
    out, _ = run(inputs, trace=False)
    return out

